# revision 1
# baseline (speedup 1.0000x reference)
"""Trainium2 Bass kernel for nn_AttentionBlock (B=8, C=512, H=W=32, 8 heads).

Sharding: data-parallel over batch — core b computes batch image b end-to-end
(attention is independent per (batch, head), so this is embarrassingly
parallel; weights are replicated to all 8 cores).

Per-core pipeline (x_b viewed as (C=512, S=1024) channels-on-partition):
  P1a: q,k = Wqk^T.T @ x          -> (1024, S) PSUM->SBUF, channel order
       arranged (on host) so each 128-row tile is one head-PAIR of q or k.
  P1b: vT  = x.T @ Wv^T           -> (S, 512) directly transposed, so no PE
       transposes are needed for attention; a ones column is appended per
       head (65 cols/head) to produce softmax denominators for free.
  P2 : scoresT[t,s] = k^T q per head; head pairs run CONCURRENTLY in the PE
       array via row tiling (K=64 each at base partitions 0/64).
  exp: ACT exp(0.125 * scoresT) PSUM->SBUF, one (128,2048) instr per t-tile.
  P3 : outT_aug[h] = [vT|1]^T @ expT  (65, S): row 64 = softmax denominator.
  norm: DVE reciprocal -> ones outer-product PE broadcast -> DVE multiply.
  P4 : y = Wo^T.T @ res + bo + x (4 concurrent PSUM accumulators) -> DMA out.

All matmuls run as float32r (fp32 bits, full-rate 1 cycle/row PE mode).
next-pair q/k projections and pair-0 vT tiles are interleaved into the
attention j-loop so the PE fills the ACT (exp) slack; tiny "corner" matmuls
and scratch copies act as semaphore-wait carriers because several walrus
instruction structs encode only a single wait (see pe_mm/dve_sync and
_strip_self_waits/_install_drain_split).
"""

import os
import sys

for _p in ("/opt/trn_rl_repo", "/root/.axon_site/_ro/trn_rl_repo"):
    if os.path.isdir(_p) and _p not in sys.path:
        sys.path.insert(0, _p)

from contextlib import ExitStack

import numpy as np

import concourse.bass as bass
import concourse.tile as tile
from concourse import mybir
from concourse.bass_utils import run_bass_kernel_spmd

B, C, H, W = 8, 512, 32, 32
NH, D = 8, 64
S = H * W            # 1024 sequence positions
P = 128              # partitions
KT = C // P          # 4 contraction tiles over channels
MT_QK = 2 * C // P   # 8 output tiles for q,k
NT = S // P          # 8 t-tiles
NPAIR = NH // 2      # 4 head pairs
DA = D + 1           # 65: v columns + ones column per head
F32 = mybir.dt.float32
AF = mybir.ActivationFunctionType
ALU = mybir.AluOpType

EXP_BUFS = int(os.environ.get("K_EXP_BUFS", "12"))
ACT_K = int(os.environ.get("K_ACT_K", "8"))
INS_J = tuple(int(c) for c in os.environ.get("K_INS_J", "0134"))
USE_F32R = os.environ.get("K_F32R", "1") == "1"


def _r(ap):
    """Matmul-operand dtype: float32r streams 1 col/cycle (vs 4 for fp32)."""
    return ap.bitcast(mybir.dt.float32r) if USE_F32R else ap


def _install_drain_split():
    """walrus's CTRL_NO (drain) codegen accepts only a single semaphore wait,
    but Tile's kernel-tail drain aggregates one wait per live proc.  Split
    them across several serial drains (semantically identical: all complete
    before the closing all-engine barrier)."""
    if getattr(tile.TileContext, "_drain_split_installed", False):
        return
    from concourse.vector_clock import ScopedClock

    orig = tile.TileContext._drain_and_barrier

    def patched(self, tick_clock, wait_clock):
        nc = self.nc
        drain_inst = nc.sync.drain()
        wait_clock.add_sem_waits(
            drain_inst.ins, ScopedClock({None: tick_clock.global_clock})
        )
        si = drain_inst.ins.sync_info
        if si is not None and si.on_wait and len(si.on_wait) > 1:
            waits = list(si.on_wait)
            drain_inst.ins.sync_info = mybir.SyncInfo(
                on_wait=[waits[0]], on_update=list(si.on_update or [])
            )
            for w in waits[1:]:
                d2 = nc.sync.drain()
                d2.ins.sync_info = mybir.SyncInfo(on_wait=[w], on_update=[])

        nc.all_engine_barrier()
        assert self.sems is not None
        popped = nc._tile_sem_poison_stack.pop()
        assert popped is self._sem_poison
        nc.clear_and_free_semaphores(list(self.sems.allocated().values()))
        nc.all_engine_barrier()

    tile.TileContext._drain_and_barrier = patched
    tile.TileContext._drain_split_installed = True
    tile.TileContext._drain_and_barrier_orig = orig


def trace_kernel(ctx, tc, nc, x, wqk, y):
    cst = ctx.enter_context(tc.tile_pool(name="cst", bufs=1))
    qkp = ctx.enter_context(tc.tile_pool(name="qkp", bufs=4))
    expp = ctx.enter_context(tc.tile_pool(name="expp", bufs=EXP_BUFS))
    resp = ctx.enter_context(tc.tile_pool(name="resp", bufs=1))
    rdp = ctx.enter_context(tc.tile_pool(name="rdp", bufs=2))
    rbp = ctx.enter_context(tc.tile_pool(name="rbp", bufs=2))
    yp = ctx.enter_context(tc.tile_pool(name="yp", bufs=1))
    pa = ctx.enter_context(tc.tile_pool(name="pa", bufs=2, space="PSUM"))
    pb = ctx.enter_context(tc.tile_pool(name="pb", bufs=2, space="PSUM"))

    xt = cst.tile([P, KT, S], F32)
    wall = cst.tile([P, KT, 2 * C + C + C + 1], F32)
    wqkt = wall[:, :, 0:2 * C]
    wvt = wall[:, :, 2 * C:2 * C + C]
    wot = wall[:, :, 3 * C:4 * C]
    ones = cst.tile([1, D], F32)
    scr = cst.tile([1, 256], F32)
    scra = cst.tile([1, 8], F32)
    vta = cst.tile([P, NT, NH * DA], F32)
    res = resp.tile([P, KT, S], F32)

    nc.sync.dma_start(out=_r(xt[:, :, :]),
                      in_=_r(x.rearrange("(k p) s -> p k s", p=P)))
    wallr = wqk.rearrange("(k p) s -> p k s", p=P)
    nc.gpsimd.dma_start(out=_r(wall[:, :, 0:256]), in_=_r(wallr[:, :, 0:256]))
    nc.gpsimd.dma_start(out=_r(wall[:, :, 256:2 * C]),
                      in_=_r(wallr[:, :, 256:2 * C]))
    nc.gpsimd.dma_start(out=_r(wall[:, :, 2 * C:]), in_=_r(wallr[:, :, 2 * C:]))

    scr_i = [0]

    def dve_sync(*aps):
        # DVE wait-carrier: absorb one cross-engine wait per tiny copy.
        # Disjoint scratch columns avoid WAW self-waits between carriers.
        for ap in aps:
            n = ap.free_size()
            o = (scr_i[0] % 30) * 8
            scr_i[0] += 1
            nc.vector.tensor_copy(scr[0:1, o:o + n], ap)
    def pe_mm(corner, dep):
        # PE wait-carrier: a 1x2 matmul reading `dep` absorbs one cross-
        # engine wait; PE program order subsumes the tick for later matmuls.
        # `corner` is a PSUM slice overwritten by the next start=True group.
        nc.tensor.matmul(
            corner, _r(dep[:, 0:1]), _r(dep[:, 0:2]),
            start=True, stop=True, skip_group_check=True,
        )

    # exp(0*x) = 1.0 writes: DVE memset can't emit float32r, ACT can
    nc.scalar.activation(_r(ones[:, :]), _r(wall[0:1, 0, 0:D]), AF.Exp, scale=0.0)
    # ones column per head in the augmented vT (softmax denominator trick)
    nc.scalar.activation(
        _r(vta.rearrange("p j (h e) -> p j h e", h=NH)[:, :, :, D:DA]),
        _r(xt[:, 0, 0:NT * NH].rearrange("p (j h) -> p j h", h=NH)[:, :, :, None]
           if False else xt[:, 0, 0:NT * NH]),
        AF.Exp, scale=0.0,
    )

    dve_sync(xt[0:1, 0, 0:4])

    # PSUM: pa's single slot (128,2048) holds score tiles; pb's two (*,1024)
    # slots rotate between P1/P4 accumulators and P3 head accumulators.
    def acc_tile(i, shape):
        return pb.tile(shape, F32, tag="ob", name=f"acc{i}")

    qk_tiles = [None] * NPAIR
    nacc = 0
    ets_hist = []

    def act_sync_maybe():
        # Batched ACT wait-carrier: exp tiles cycle through EXP_BUFS slots;
        # each reuse makes the next exp wait on the slot's previous ACT
        # writer.  One cheap ACT copy pre-waiting on a newer tick covers the
        # next ACT_K reuses (the ACT semaphore is monotonic).
        n = len(ets_hist)
        if n >= EXP_BUFS and (n - EXP_BUFS) % ACT_K == 0:
            nc.scalar.copy(scra[0:1, 0:2], ets_hist[n - EXP_BUFS + ACT_K][0:1, 0:2])

    def p1a_mtile(m):
        nonlocal nacc
        pair, isk = divmod(m, 2)
        if isk == 0:
            qk_tiles[pair] = qkp.tile([P, 2 * S], F32, tag="qk", name=f"qk{pair}")
        acc = acc_tile(nacc, [P, S])
        nacc += 1
        if m == 0:
            pe_mm(acc[0:1, 0:2], wall[0:1, 0, 0:2])
        for n in range(2):
            for k in range(KT):
                nc.tensor.matmul(
                    acc[:, n * 512:(n + 1) * 512],
                    _r(wqkt[:, k, m * P:(m + 1) * P]),
                    _r(xt[:, k, n * 512:(n + 1) * 512]),
                    start=(k == 0),
                    stop=(k == KT - 1),
                )
        dve_sync(acc[0:1, 508:516])
        nc.vector.tensor_copy(
            _r(qk_tiles[pair][:, isk * S:(isk + 1) * S]), _r(acc[:, :])
        )

    def p1a_half(m, n, sync_ap=None):
        # half an m-tile (one 512-column n-slice) through a pa slot: small
        # enough to hide inside the attention j-loop's ACT slack
        pair, isk = divmod(m, 2)
        if isk == 0 and n == 0:
            qk_tiles[pair] = qkp.tile([P, 2 * S], F32, tag="qk", name=f"qk{pair}")
        acc = pa.tile([P, 512], F32, tag="sc", name=f"acc{m}_{n}")
        if m == 2 and n == 0 and sync_ap is not None:
            pe_mm(acc[0:1, 0:2], sync_ap)
            pe_mm(acc[0:1, 0:2], wall[0:1, 0, 256:258])
        if m == 0 and n == 0:
            pe_mm(acc[0:1, 0:2], wall[0:1, 0, 0:2])
        for k in range(KT):
            nc.tensor.matmul(
                acc[:, :],
                _r(wqkt[:, k, m * P:(m + 1) * P]),
                _r(xt[:, k, n * 512:(n + 1) * 512]),
                start=(k == 0),
                stop=(k == KT - 1),
            )
        dve_sync(acc[0:1, 252:260])
        nc.vector.tensor_copy(
            _r(qk_tiles[pair][:, isk * S + n * 512: isk * S + (n + 1) * 512]),
            _r(acc[:, :]),
        )

    def p1b_jtile(j, ets=None):
        # Lives in the pa pool: inside pair 0's j-loop both pb slots are
        # held by the oa accumulators.
        acc = pa.tile([P, C], F32, tag="sc", name=f"vacc{j}")
        if j == 0:
            pe_mm(acc[0:1, 0:2], ets[0][0:1, 0:2])
            pe_mm(acc[0:1, 0:2], wall[0:1, 0, 2 * C:2 * C + 2])
        for k in range(KT):
            nc.tensor.matmul(
                acc[:, :],
                _r(xt[:, k, j * P:(j + 1) * P]),
                _r(wvt[:, k, :]),
                start=(k == 0),
                stop=(k == KT - 1),
            )
        nc.vector.tensor_copy(
            _r(vta[:, j, :].rearrange("p (h e) -> p h e", h=NH)[:, :, 0:D]),
            _r(acc.rearrange("p (h d) -> p h d", h=NH)),
        )
        return acc

    def fused_pair(pair, norm_prev=None):
        """Per t-tile: scoresT matmuls -> exp -> attn@v accumulate.

        The P3 accumulation for t-tile j consumes exp tile j right away, so
        only EXP_BUFS exp tiles are ever live.  Pair 0 additionally computes
        vT (p1b) tile j inside round j — P3 only needs vta[:, j, :].
        """
        qk = qk_tiles[pair]
        oa = None
        for j in range(NT):
            ets, scs = [], []
            for hh in range(2):
                act_sync_maybe()
                et = expp.tile([P, S], F32, tag="et", name=f"et{pair}_{j}_{hh}")
                ets_hist.append(et)
                sc = pa.tile([P, S], F32, tag="sc", name=f"sc{pair}_{j}_{hh}")
                scs.append(sc)
                for n in range(2):
                    nc.tensor.matmul(
                        sc[:, n * 512:(n + 1) * 512],
                        _r(qk[64 * hh:64 * (hh + 1), S + j * P: S + (j + 1) * P]),
                        _r(qk[64 * hh:64 * (hh + 1), n * 512:(n + 1) * 512]),
                        start=True,
                        stop=True,
                    )
                nc.scalar.activation(
                    _r(et[:, :]), _r(sc[:, :]), AF.Exp, scale=1.0 / np.sqrt(D)
                )
                ets.append(et)
            if j == 0:
                if norm_prev is not None:
                    norm_prev()
                oa = [
                    pb.tile([DA, S], F32, tag="ob", name=f"oa{pair}_{hh}")
                    for hh in range(2)
                ]
                if pair == 0:
                    pe_mm(oa[0][0:1, 0:2], qk[0:1, S:S + 2])
                else:
                    pe_mm(oa[0][0:1, 0:2], res[64:65, pair - 1, 0:2])
            if pair == 0:
                vacc = p1b_jtile(j, ets)
                # DVE tick (vta j) rides on the dead vacc corner: its WAR
                # against the vacc evict is on the same DVE semaphore
                pe_mm(vacc[0:1, 0:2], vta[0:1, j, 0:2])
            ins_j = (4, 5, 6, 7) if pair == 0 else INS_J
            if pair < NPAIR - 1 and j in ins_j:
                mm_ = 2 * (pair + 1) + (j >= ins_j[2])
                p1a_half(mm_, 0 if j in (ins_j[0], ins_j[2]) else 1,
                         sync_ap=ets[0][0:1, 0:2])
            for hh in range(2):
                h = 2 * pair + hh
                for n in range(2):
                    nc.tensor.matmul(
                        oa[hh][:, n * 512:(n + 1) * 512],
                        _r(vta[:, j, h * DA:(h + 1) * DA]),
                        _r(ets[hh][:, n * 512:(n + 1) * 512]),
                        start=(j == 0),
                        stop=(j == NT - 1),
                        skip_group_check=True,
                    )
        def do_norm():
            for hh in range(2):
                rd = rdp.tile([1, S], F32, tag="rd", name=f"rd{pair}_{hh}")
                with nc.allow_low_precision(reason="f32r view of reciprocal"):
                    nc.vector.reciprocal(_r(rd[:, :]), oa[hh][D:DA, :])
                # broadcast 1/denom across the 64 head channels: ones (1,64)
                # outer-product matmul, then evict and multiply on DVE
                bc = pa.tile([D, S], F32, tag="sc", name=f"bc{pair}_{hh}")
                pe_mm(bc[0:1, 0:2], ets_hist[-2 + hh][0:1, 0:2])
                pe_mm(bc[0:1, 0:2], rd[0:1, 0:2])
                for n in range(2):
                    nc.tensor.matmul(
                        bc[:, n * 512:(n + 1) * 512],
                        _r(ones[:, :]),
                        _r(rd[:, n * 512:(n + 1) * 512]),
                        start=True,
                        stop=True,
                    )
                rb = rbp.tile([D, S], F32, tag="rb", name=f"rb{pair}_{hh}")
                nc.vector.tensor_copy(rb[:, :], bc[:, :])
                nc.vector.tensor_mul(
                    _r(res[64 * hh:64 * (hh + 1), pair, :]),
                    _r(oa[hh][0:D, :]), _r(rb[:, :]),
                )
        return do_norm

    # ---- schedule trace ----
    p1a_half(0, 0)
    p1a_half(1, 0)
    p1a_half(0, 1)
    p1a_half(1, 1)
    norm_prev = None
    for pair in range(NPAIR):
        norm_prev = fused_pair(pair, norm_prev)
    norm_prev()

    dve_sync(xt[0:1, 0, 4:8], wall[0:1, 0, 4 * C:4 * C + 1])
    ybig = yp.tile([P, KT, S], F32, tag="y", name="yb")
    for m in range(KT):
        if m >= 2:
            acc = pa.tile([P, S], F32, tag="sc", name=f"p4acc{m}")
        else:
            acc = acc_tile(m, [P, S])
        if m == 0:
            pe_mm(acc[0:1, 0:2], res[64:65, NPAIR - 1, 0:2])
        elif m == 1:
            pe_mm(acc[0:1, 0:2], res[0:1, NPAIR - 1, 0:2])
        for n in range(2):
            for k in range(KT):
                nc.tensor.matmul(
                    acc[:, n * 512:(n + 1) * 512],
                    _r(wot[:, k, m * P:(m + 1) * P]),
                    _r(res[:, k, n * 512:(n + 1) * 512]),
                    start=(k == 0),
                    stop=(k == KT - 1),
                )
        dve_sync(acc[0:1, 508:516])
        nc.vector.scalar_tensor_tensor(
            _r(ybig[:, m, :]), acc[:, :], wall[:, m, 4 * C:4 * C + 1],
            xt[:, m, :], op0=ALU.add, op1=ALU.add,
        )
        if m == 1 or m == KT - 1:
            yr = y.rearrange("(k p) s -> p k s", p=P)
            nc.gpsimd.dma_start(
                out=yr[:, m - 1:m + 1, :], in_=ybig[:, m - 1:m + 1, :]
            )


ENGINE_SEM_PREFIX = {
    "PE": "PE_",
    "Activation": "Activation_",
    "DVE": "DVE_",
    "Pool": "Pool_",
    "SP": "SP_",
}


def _strip_self_waits(nc):
    """Drop same-engine semaphore self-waits from multi-wait instructions.

    Engines execute and complete their own instructions in program order
    (PE matmuls are pc-monotone in start and end; ACT/DVE/Pool are strict
    FIFO with per-op drains), so a wait on the engine's own completion
    semaphore is redundant whenever the instruction carries another wait —
    and walrus's PE/ACT instruction structs only encode a single wait.
    """
    n = 0
    for inst in nc.inst_map.values():
        si = getattr(inst, "sync_info", None)
        if si is None or not si.on_wait or len(si.on_wait) <= 1:
            continue
        eng = str(getattr(inst, "engine", "")).split(".")[-1]
        pref = ENGINE_SEM_PREFIX.get(eng)
        if pref is None:
            continue
        keep = [w for w in si.on_wait if not w.ant_name.startswith(pref)]
        if len(keep) != len(si.on_wait) and keep:
            inst.sync_info = mybir.SyncInfo(
                on_wait=keep, on_update=list(si.on_update or [])
            )
            n += 1
    return n


def build_nc():
    _install_drain_split()
    nc = bass.Bass(trn_type="TRN2", debug=False, num_devices=8)
    x_d = nc.dram_tensor("x", [C, S], F32, kind="ExternalInput")
    wqk_d = nc.dram_tensor("wqkt", [C, 4 * C + 1], F32, kind="ExternalInput")
    y_d = nc.dram_tensor("y", [C, S], F32, kind="ExternalOutput")
    with tile.TileContext(nc) as tc, ExitStack() as ctx:
        trace_kernel(ctx, tc, nc, x_d.ap(), wqk_d.ap(), y_d.ap())
    _strip_self_waits(nc)
    if not nc.is_finalized():
        nc.finalize()
    return nc


def host_inputs(x, Wqkv, Wo, bo):
    """Host-side reshard: per-core input dicts (weights replicated)."""
    x = np.ascontiguousarray(np.asarray(x, dtype=np.float32))
    Wqkv = np.asarray(Wqkv, dtype=np.float32)
    Wo = np.asarray(Wo, dtype=np.float32)
    bo = np.asarray(bo, dtype=np.float32)

    # Wqkv rows per head h: [h*3D, h*3D+D) = q, [+D, +2D) = k, [+2D, +3D) = v.
    # q,k channel order: per pair -> [q(2p)|q(2p+1)], [k(2p)|k(2p+1)] tiles.
    order = []
    for p in range(NPAIR):
        for h in (2 * p, 2 * p + 1):
            order.extend(range(h * 3 * D, h * 3 * D + D))          # q rows
        for h in (2 * p, 2 * p + 1):
            order.extend(range(h * 3 * D + D, h * 3 * D + 2 * D))  # k rows
    wqkt = Wqkv[order].T                                            # (C, 2C)
    v_order = [h * 3 * D + 2 * D + d for h in range(NH) for d in range(D)]
    wvt = Wqkv[v_order].T                                           # (C, C)
    wot = Wo.T                                                      # (C, C)
    wall = np.ascontiguousarray(
        np.concatenate([wqkt, wvt, wot, bo[:, None]], axis=1)
    )                                                               # (C, 4C+1)

    return [
        dict(x=np.ascontiguousarray(x[b].reshape(C, S)), wqkt=wall)
        for b in range(B)
    ]


_NC_CACHE = []

try:
    # bass_exec HLO does not embed the BIR; bust jax's executable cache so a
    # rebuilt kernel is actually recompiled instead of hitting a stale NEFF.
    import jax as _jax

    _jax.clear_caches()
except Exception:
    pass


def get_nc():
    if not _NC_CACHE:
        _NC_CACHE.append(build_nc())
    return _NC_CACHE[0]


def run(in_maps, **kwargs):
    return run_bass_kernel_spmd(get_nc(), in_maps, core_ids=list(range(B)), **kwargs)


def kernel(x, Wqkv, Wo, bo):
    in_maps = host_inputs(x, Wqkv, Wo, bo)
    r = run(in_maps)
    y = np.stack([r.results[b]["y"].reshape(C, H, W) for b in range(B)])
    return y.astype(np.float32)


if __name__ == "__main__":
    nc = build_nc()
    print("built ok:", len(nc.inst_map), "instructions")



# revision 2
# speedup vs baseline: 1.3372x; 1.3372x over previous
"""Trainium2 Bass kernel v3 for nn_AttentionBlock (B=8, C=512, H=W=32, 8 heads).

Sharding: data-parallel over batch (core b owns image b; weights replicated).

All heavy matmuls run as fp8e4m3 DoubleRow (0.5 cycles/row, 2 stacked
K-planes per instruction); softmax exp runs on ACT with fp8 output and a
constant -1 logit shift (softmax-invariant) so exp() fits e4m3 range.

Per-core pipeline (x viewed as (C=512, S=1024)):
  P1qk: W-piece DR matmuls emit q,k d-SPLIT: q8/k8 (64p, 2, S) fp8 where
        partition p<32 is head A, 32<=p<64 head B, plane i = d-half.  This
        costs 2x the minimal P1 instruction count but enables...
  P2  : scores DR: lhsT=k8 (K=32, planes=d-halves) -> scoresT (128t, S) psum,
        at 0.5 cyc/row with no repacking.
  exp : ACT exp(0.125*sc - 1) -> et8 fp8 tiles (128t, 2, S); the 2-plane
        j-PAIR layout feeds attn@v DR directly.
  P3  : attn@v TALL: out(s-block 128, 65) = et8^T @ [v|1]: the 65-col output
        orientation costs out-free=65 per instruction and lands the softmax
        denominator as a per-partition COLUMN (col 64).
  norm: DVE reciprocal (free=4) + stride-0-broadcast tensor_tensor multiply
        -> res8T (s-part) fp8; PE fp8 transposes (via identity) flip it back
        to channel-partitions for...
  P4  : output projection DR + bias + residual (DVE stt) -> DMA out.
"""

import os
import sys

for _p in ("/opt/trn_rl_repo", "/root/.axon_site/_ro/trn_rl_repo"):
    if os.path.isdir(_p) and _p not in sys.path:
        sys.path.insert(0, _p)

from contextlib import ExitStack

import numpy as np
import ml_dtypes

import concourse.bass as bass
import concourse.tile as tile
from concourse import mybir
from concourse.bass_utils import run_bass_kernel_spmd

B, C, H, W = 8, 512, 32, 32
NH, D = 8, 64
S = H * W            # 1024
P = 128
NPAIR = NH // 2      # 4
NT = 8               # t-tiles (128 each)
NJJ = 4              # j-pairs
NSB = 8              # s-blocks for tall attn@v
DA = D + 1           # 65 = v cols + ones col
OQK, OV, OWO = 0, 1024, 1536   # w8 column sections
OTOT = 2048
F32 = mybir.dt.float32
F8 = mybir.dt.float8e4
AF = mybir.ActivationFunctionType
ALU = mybir.AluOpType
PM = mybir.MatmulPerfMode
EXP_SHIFT = -1.0
EXP_SCALE = 1.0 / np.sqrt(D)
DEBUG_DUMP = os.environ.get("K3_DEBUG", "0") == "1"


def _install_drain_split():
    """walrus's CTRL_NO (drain) codegen accepts only a single semaphore wait,
    but Tile's kernel-tail drain aggregates one wait per live proc.  Split
    them across several serial drains."""
    if getattr(tile.TileContext, "_drain_split_installed", False):
        return
    from concourse.vector_clock import ScopedClock

    orig = tile.TileContext._drain_and_barrier

    def patched(self, tick_clock, wait_clock):
        nc = self.nc
        drain_inst = nc.sync.drain()
        wait_clock.add_sem_waits(
            drain_inst.ins, ScopedClock({None: tick_clock.global_clock})
        )
        si = drain_inst.ins.sync_info
        if si is not None and si.on_wait and len(si.on_wait) > 1:
            waits = list(si.on_wait)
            drain_inst.ins.sync_info = mybir.SyncInfo(
                on_wait=[waits[0]], on_update=list(si.on_update or [])
            )
            for w in waits[1:]:
                d2 = nc.sync.drain()
                d2.ins.sync_info = mybir.SyncInfo(on_wait=[w], on_update=[])

        nc.all_engine_barrier()
        assert self.sems is not None
        popped = nc._tile_sem_poison_stack.pop()
        assert popped is self._sem_poison
        nc.clear_and_free_semaphores(list(self.sems.allocated().values()))
        nc.all_engine_barrier()

    tile.TileContext._drain_and_barrier = patched
    tile.TileContext._drain_split_installed = True
    tile.TileContext._drain_and_barrier_orig = orig


ENGINE_SEM_PREFIX = {
    "PE": "PE_",
    "Activation": "Activation_",
    "DVE": "DVE_",
    "Pool": "Pool_",
    "SP": "SP_",
}


def _strip_self_waits(nc):
    """Drop same-engine semaphore self-waits from multi-wait instructions
    (engines complete their own instructions in program order)."""
    n = 0
    for inst in nc.inst_map.values():
        si = getattr(inst, "sync_info", None)
        if si is None or not si.on_wait or len(si.on_wait) <= 1:
            continue
        eng = str(getattr(inst, "engine", "")).split(".")[-1]
        pref = ENGINE_SEM_PREFIX.get(eng)
        if pref is None:
            continue
        keep = [w for w in si.on_wait if not w.ant_name.startswith(pref)]
        if len(keep) != len(si.on_wait) and keep:
            inst.sync_info = mybir.SyncInfo(
                on_wait=keep, on_update=list(si.on_update or [])
            )
            n += 1
    return n


def trace_kernel(ctx, tc, nc, x, x8d, w8d, id8d, bofd, y, dbg=None):
    cst = ctx.enter_context(tc.tile_pool(name="cst", bufs=1))
    qkp = ctx.enter_context(tc.tile_pool(name="qkp", bufs=4))
    expp = ctx.enter_context(tc.tile_pool(name="expp", bufs=16))
    rstp = ctx.enter_context(tc.tile_pool(name="rstp", bufs=2))
    rdp = ctx.enter_context(tc.tile_pool(name="rdp", bufs=2))
    yp = ctx.enter_context(tc.tile_pool(name="yp", bufs=1))
    rot = ctx.enter_context(tc.tile_pool(name="rot", bufs=2, space="PSUM"))
    spp = ctx.enter_context(tc.tile_pool(name="spp", bufs=1, space="PSUM"))
    otp = ctx.enter_context(tc.tile_pool(name="otp", bufs=1, space="PSUM"))

    xt = cst.tile([P, 4, S], F32)
    x8 = cst.tile([P, 2, 2, S], F8)
    w8 = cst.tile([P, 2, 2, OTOT], F8)
    id8 = cst.tile([P, P], F8)
    bof = cst.tile([P, 5], F32)
    v8a = cst.tile([P, NJJ, 2, NH * DA], F8)
    res8 = cst.tile([P, 2, 2, S], F8)
    scr = cst.tile([1, 256], F32)
    scr8 = cst.tile([1, 64], F32)
    ybig = yp.tile([P, 4, S], F32)

    w8a_d, w8b_d, w8v_d, w8o_d = w8d
    nc.gpsimd.dma_start(out=w8[:, :, :, 0:256], in_=w8a_d)
    nc.sync.dma_start(out=x8[:, :, :, :], in_=x8d)
    nc.gpsimd.dma_start(out=bof[:, :], in_=bofd)
    nc.gpsimd.dma_start(out=w8[:, :, :, OV:OV + 512], in_=w8v_d)
    nc.sync.dma_start(out=w8[:, :, :, 256:OV], in_=w8b_d)
    nc.gpsimd.dma_start(out=w8[:, :, :, OWO:OTOT], in_=w8o_d)
    nc.gpsimd.dma_start(out=id8[:, :], in_=id8d)
    nc.sync.dma_start(out=xt[:, :, :], in_=x.rearrange("(k p) s -> p k s", p=P))

    scr_i = [0]

    def dve_sync(*aps):
        # DVE wait-carrier: absorb one cross-engine wait per tiny copy.
        for ap in aps:
            n = min(ap.free_size(), 8)
            o = (scr_i[0] % 30) * 8
            scr_i[0] += 1
            nc.vector.tensor_copy(scr[0:1, o:o + n], ap)

    def pe_mm(corner, dep):
        # PE wait-carrier: a 1x2 matmul reading `dep` absorbs one cross-
        # engine wait; PE program order subsumes the tick for later matmuls.
        nc.tensor.matmul(
            corner, dep[:, 0:1], dep[:, 0:2],
            start=True, stop=True, skip_group_check=True,
        )

    # ones columns of v8a (fp8 1.0 via exp(0*x)), one ACT instr per jj;
    # reading bof also absorbs its DMA tick into the ACT clock (the exp
    # instrs' bias operand then needs no extra wait)
    # tiny ACT read of bof first: absorbs its DMA tick into the ACT clock
    dum = cst.tile([P, 1], F32)
    nc.scalar.activation(dum[:, :], bof[:, 4:5], AF.Exp, scale=0.0)
    for jj in range(NJJ):
        nc.scalar.activation(
            v8a.rearrange("p j i (h e) -> p j i h e", h=NH)[:, jj, :, :, D:DA],
            x8[:, 0, :, jj * NH:(jj + 1) * NH][:, :, :, None],
            AF.Exp, scale=0.0,
        )

    # ---------------- P1: v projection (tall: t on partitions) -------------
    # v8a[p, jj, i, h*65+e] ; v-acc psum (128t, 512 vchans)
    def p1v_jtile(j):
        acc = otp.tile([P, 512], F32, tag="ot", name=f"vacc{j}")
        if j == 0:
            pe_mm(acc[0:1, 0:2], w8[0:1, 0, 0, OV:OV + 2])
        for kk in range(2):
            nc.tensor.matmul(
                acc[:, :],
                x8[:, kk, :, j * P:(j + 1) * P],
                w8[:, kk, :, OV:OV + 512],
                start=(kk == 0), stop=(kk == 1),
                perf_mode=PM.DoubleRow,
            )
        nc.vector.tensor_copy(
            v8a[:, j // 2, j % 2, :].rearrange("p (h e) -> p h e", h=NH)[:, :, 0:D],
            acc.rearrange("p (h d) -> p h d", h=NH),
        )

    # ---------------- P1: q/k pieces (d-split, M=64) ------------------------
    qk_tiles = {}

    def p1qk_piece(pair, tqk, half):
        """One M=64 piece: rows = [head 2pair (32) | head 2pair+1 (32)] of
        q-half or k-half `half`; accumulates 2 DR k-steps; evicts to fp8."""
        key = (pair, tqk)
        if key not in qk_tiles:
            qk_tiles[key] = qkp.tile(
                [D, 2, S], F8, tag="qk", name=f"qk{pair}_{tqk}")
        t8 = qk_tiles[key]
        off = OQK + pair * 256 + (0 if tqk == 0 else 128) + half * D
        acc = spp.tile([D, S], F32, tag="sp", name=f"qkacc{pair}_{tqk}_{half}")
        if pair == 0 and tqk == 0 and half == 0:
            pe_mm(acc[0:1, 0:2], w8[0:1, 0, 0, 0:2])
            pe_mm(acc[0:1, 0:2], x8[0:1, 0, 0, 0:2])
        for n in range(2):
            for kk in range(2):
                nc.tensor.matmul(
                    acc[:, n * 512:(n + 1) * 512],
                    w8[:, kk, :, off:off + D],
                    x8[:, kk, :, n * 512:(n + 1) * 512],
                    start=(kk == 0), stop=(kk == 1),
                    perf_mode=PM.DoubleRow,
                )
        nc.vector.tensor_copy(t8[:, half, :], acc[:, :])

    # ---------------- attention pair ---------------------------------------
    # Scores+exp stream h2-outer (all head-A j's, then head-B); the pair's
    # attn@v GROUPS (one (128,65) psum bank per (h2,sb), 4 DR jj-steps +
    # DVE recip + DVE normalize-mul) are deferred until all its exp tiles
    # exist and run as spare work inside the NEXT pair's slots.  PSUM
    # accumulation groups zero their whole 2KB bank on start, so concurrent
    # groups must own a bank: the 16 groups ping-pong through 2 banks.
    def fused_pair(pair, spare, self_h0=False):
        q8 = qk_tiles[(pair, 0)]
        k8 = qk_tiles[(pair, 1)]
        ets = {}
        slot = [0]

        def run_spare():
            s = slot[0]
            slot[0] += 1
            if s < len(spare):
                for thunk in spare[s]:
                    thunk()
            if self_h0 and 8 <= s < 16:
                k = s - 8
                group(0, k, first=(k == 0))

        res8T = rstp.tile([P, NSB, 2, D], F8, tag="rt", name=f"r8t{pair}")
        rd = rdp.tile([P, 16], F32, tag="rd", name=f"rd{pair}")

        def group(h2, sb, first=False):
            h = 2 * pair + h2
            g = h2 * NSB + sb
            ot = otp.tile([P, DA], F32, tag="ot", name=f"ot{pair}_{g}")
            if first:
                # absorb the ACT tick of this pair's last relevant exp
                pe_mm(ot[0:1, 0:2], ets[(NJJ - 1, h2)][0:1, 1, 0:2])
            for jj in range(NJJ):
                nc.tensor.matmul(
                    ot[:, :],
                    ets[(jj, h2)][:, :, sb * P:(sb + 1) * P],
                    v8a[:, jj, :, h * DA:(h + 1) * DA],
                    start=(jj == 0), stop=(jj == NJJ - 1),
                    perf_mode=PM.DoubleRow,
                    skip_group_check=True,
                )
            nc.vector.reciprocal(rd[:, g:g + 1], ot[:, D:DA])
            rdb = rd[:, g:g + 1]
            rdb = bass.AP(rdb.tensor, rdb.offset, rdb.ap[:-1] + [[0, D]])
            nc.vector.tensor_tensor(
                out=res8T[:, sb, h2, :], in0=ot[:, 0:D], in1=rdb,
                op=ALU.mult)

        def transp():
            # transposes: (128s,128c)->psum fp8 (step 2); evict to res8
            tp = spp.tile([P, 2 * S], F8, tag="sp", name=f"tp{pair}")
            tpv = tp.rearrange("p (n two) -> p n two", two=2)[:, :, 0]
            for sb in range(NSB):
                nc.tensor.matmul(
                    tpv[:, sb * P:(sb + 1) * P],
                    res8T[:, sb, :, :], id8[:, :],
                    start=True, stop=True, is_transpose=True,
                    skip_group_check=True,
                )
            nc.vector.tensor_copy(res8[:, pair // 2, pair % 2, :], tpv[:, :])

        for h2 in range(2):
            for j in range(NT):
                jj, jhalf = j // 2, j % 2
                if jhalf == 0:
                    ets[(jj, h2)] = expp.tile(
                        [P, 2, S], F8, tag="et", name=f"et{pair}_{jj}_{h2}")
                sc = rot.tile([P, S], F32, tag="sc", name=f"sc{pair}_{j}_{h2}")
                if j == 0 and h2 == 0:
                    # absorb DVE tick of this pair's q8/k8 evicts
                    pe_mm(sc[0:1, 0:2], k8[0:1, 1, 0:2])
                for n in range(2):
                    nc.tensor.matmul(
                        sc[:, n * 512:(n + 1) * 512],
                        k8[32 * h2:32 * h2 + 32, :, j * P:(j + 1) * P],
                        q8[32 * h2:32 * h2 + 32, :, n * 512:(n + 1) * 512],
                        start=True, stop=True,
                        perf_mode=PM.DoubleRow,
                    )
                nc.scalar.activation(
                    ets[(jj, h2)][:, jhalf, :], sc[:, :], AF.Exp,
                    bias=bof[:, 4:5], scale=EXP_SCALE,
                )
                run_spare()

        out = []
        h2s = [1] if self_h0 else [0, 1]
        for h2 in h2s:
            for sb in range(NSB):
                out.append(lambda h2=h2, sb=sb,
                           f=(h2 == h2s[0] and sb == 0 and not self_h0):
                           group(h2, sb, first=f))
        out.append(transp)
        return out

    # ---------------- schedule ---------------------------------------------
    # prologue: pair-0 q/k pieces + first v tiles
    p1qk_piece(0, 0, 0)
    p1qk_piece(0, 0, 1)
    p1qk_piece(0, 1, 0)
    p1qk_piece(0, 1, 1)

    def mk_slots(n):
        return [[] for _ in range(n)]

    # pair 0 spare: v tiles + pair-1 pieces
    sp0 = mk_slots(16)
    for j in range(NT):
        sp0[j].append(lambda j=j: p1v_jtile(j))
    for i in range(4):
        sp0[8 + i].append(lambda i=i: p1qk_piece(1, i // 2, i % 2))

    def id8_warm():
        # dummy transpose: absorbs the id8 DMA tick into the PE clock so
        # real transposes carry only their single DVE wait
        td = otp.tile([P, 2 * P], F8, tag="ot", name="tdum")
        tdv = td.rearrange("p (n two) -> p n two", two=2)[:, :, 0]
        nc.tensor.matmul(tdv, id8[:, :], id8[:, :],
                         start=True, stop=True, is_transpose=True,
                         skip_group_check=True)
    sp0[12].append(id8_warm)
    fin0 = fused_pair(0, sp0)

    # pair 1: pair-0 finish work (16 groups + transp) + pair-2 pieces
    sp1 = mk_slots(16)
    for k in range(16):
        sp1[k].append(fin0[k])
    sp1[15].append(fin0[16])
    for i in range(4):
        sp1[2 + 3 * (i % 2) + (i // 2)].append(
            lambda i=i: p1qk_piece(2, i // 2, i % 2))
    fin1 = fused_pair(1, sp1)

    sp2 = mk_slots(16)
    for k in range(16):
        sp2[k].append(fin1[k])
    sp2[15].append(fin1[16])
    for i in range(4):
        sp2[2 + 3 * (i % 2) + (i // 2)].append(
            lambda i=i: p1qk_piece(3, i // 2, i % 2))
    fin2 = fused_pair(2, sp2)

    # pair 3: pair-2 finish in slots 0..7 (2/slot); pair-3 head-A groups
    # self-schedule into slots 8..15 once its h0 exps are done
    sp3 = mk_slots(16)
    for k in range(16):
        sp3[k // 2].append(fin2[k])
    sp3[7].append(fin2[16])
    fin3 = fused_pair(3, sp3, self_h0=True)
    # tail: pair-3 head-B groups + its transposes
    for thunk in fin3:
        thunk()
    if dbg is not None:
        nc.sync.dma_start(out=dbg["q8"], in_=qk_tiles[(0, 0)][:, :, :])
        nc.sync.dma_start(out=dbg["k8"], in_=qk_tiles[(0, 1)][:, :, :])
        nc.sync.dma_start(out=dbg["v8a"], in_=v8a[:, :, :, :])
        nc.sync.dma_start(out=dbg["res8"], in_=res8[:, :, :, :])

    # ---------------- P4 + bias + residual + DMA out ------------------------
    # absorb xt/bof DMA ticks into the DVE clock (plain copies tolerate
    # multi-waits; TensorScalarPtr does not)
    dve_sync(xt[0:1, 0, 0:8], bof[0:1, 0:4])
    for m in range(4):
        acc = rot.tile([P, S], F32, tag="sc", name=f"p4acc{m}")
        if m == 0:
            pe_mm(acc[0:1, 0:2], res8[0:1, 1, 1, 0:2])
        for n in range(2):
            for kk in range(2):
                nc.tensor.matmul(
                    acc[:, n * 512:(n + 1) * 512],
                    w8[:, kk, :, OWO + m * P:OWO + (m + 1) * P],
                    res8[:, kk, :, n * 512:(n + 1) * 512],
                    start=(kk == 0), stop=(kk == 1),
                    perf_mode=PM.DoubleRow,
                )
        nc.vector.scalar_tensor_tensor(
            ybig[:, m, :], acc[:, :], bof[:, m:m + 1],
            xt[:, m, :], op0=ALU.add, op1=ALU.add,
        )
        yr = y.rearrange("(k p) s -> p k s", p=P)
        if m % 2 == 0:
            nc.gpsimd.tensor_copy(scr8[0:1, 0:8], ybig[0:1, m, 0:8])
            nc.gpsimd.dma_start(out=yr[:, m:m + 1, :], in_=ybig[:, m:m + 1, :])
        else:
            nc.sync.dma_start(out=yr[:, m:m + 1, :], in_=ybig[:, m:m + 1, :])


def build_nc():
    _install_drain_split()
    nc = bass.Bass(trn_type="TRN2", debug=False, num_devices=8)
    x_d = nc.dram_tensor("x", [C, S], F32, kind="ExternalInput")
    x8_d = nc.dram_tensor("x8", [P, 2, 2, S], F8, kind="ExternalInput")
    w8a_d = nc.dram_tensor("w8a", [P, 2, 2, 256], F8, kind="ExternalInput")
    w8b_d = nc.dram_tensor("w8b", [P, 2, 2, 768], F8, kind="ExternalInput")
    w8v_d = nc.dram_tensor("w8v", [P, 2, 2, 512], F8, kind="ExternalInput")
    w8o_d = nc.dram_tensor("w8o", [P, 2, 2, 512], F8, kind="ExternalInput")
    id8_d = nc.dram_tensor("id8", [P, P], F8, kind="ExternalInput")
    bof_d = nc.dram_tensor("bof", [P, 5], F32, kind="ExternalInput")
    y_d = nc.dram_tensor("y", [C, S], F32, kind="ExternalOutput")
    dbg = None
    if DEBUG_DUMP:
        dbg = {
            "q8": nc.dram_tensor("dbg_q8", [D, 2, S], F8, kind="ExternalOutput").ap(),
            "k8": nc.dram_tensor("dbg_k8", [D, 2, S], F8, kind="ExternalOutput").ap(),
            "v8a": nc.dram_tensor("dbg_v8a", [P, NJJ, 2, NH * DA], F8, kind="ExternalOutput").ap(),
            "res8": nc.dram_tensor("dbg_res8", [P, 2, 2, S], F8, kind="ExternalOutput").ap(),
        }
    with tile.TileContext(nc) as tc, ExitStack() as ctx:
        trace_kernel(ctx, tc, nc, x_d.ap(), x8_d.ap(),
                     (w8a_d.ap(), w8b_d.ap(), w8v_d.ap(), w8o_d.ap()),
                     id8_d.ap(), bof_d.ap(), y_d.ap(), dbg)
    _strip_self_waits(nc)
    if not nc.is_finalized():
        nc.finalize()
    return nc


def host_inputs(x, Wqkv, Wo, bo):
    """Host-side reshard + fp8 quantization (weights replicated)."""
    f8 = ml_dtypes.float8_e4m3fn
    x = np.ascontiguousarray(np.asarray(x, dtype=np.float32))
    Wqkv = np.asarray(Wqkv, dtype=np.float32)
    Wo = np.asarray(Wo, dtype=np.float32)
    bo = np.asarray(bo, dtype=np.float32)

    # Wqkv rows per head h: [h*192, +64) = q, [+64, +128) = k, [+128, +192) = v
    wbig = np.empty((OTOT, C), dtype=np.float32)
    for pair in range(NPAIR):
        hA, hB = 2 * pair, 2 * pair + 1
        base = pair * 256
        for tqk, roff in ((0, 0), (1, D)):          # q rows, then k rows
            for half in range(2):
                o = base + tqk * 128 + half * D
                rA = hA * 192 + roff + half * 32
                rB = hB * 192 + roff + half * 32
                wbig[o:o + 32] = Wqkv[rA:rA + 32]
                wbig[o + 32:o + 64] = Wqkv[rB:rB + 32]
    for h in range(NH):
        wbig[OV + h * D:OV + (h + 1) * D] = Wqkv[h * 192 + 128:h * 192 + 192]
    wbig[OWO:OWO + C] = Wo
    # w8[p, kk, i, o] = wbig[o, (2kk+i)*128+p], shipped as 4 contiguous
    # section tensors so each DMA sprays across all engines
    w8 = wbig.T.reshape(2, 2, P, OTOT).transpose(2, 0, 1, 3).astype(f8)
    w8a = np.ascontiguousarray(w8[:, :, :, 0:256])
    w8b = np.ascontiguousarray(w8[:, :, :, 256:OV])
    w8v = np.ascontiguousarray(w8[:, :, :, OV:OV + 512])
    w8o = np.ascontiguousarray(w8[:, :, :, OWO:OTOT])
    id8 = np.eye(P, dtype=np.float32).astype(f8)
    bof = np.concatenate(
        [bo.reshape(4, P).T, np.full((P, 1), EXP_SHIFT, np.float32)], axis=1)
    bof = np.ascontiguousarray(bof)

    ins = []
    for b in range(B):
        xs = np.ascontiguousarray(x[b].reshape(C, S))
        x8 = np.ascontiguousarray(
            xs.reshape(2, 2, P, S).transpose(2, 0, 1, 3)).astype(f8)
        ins.append(dict(x=xs, x8=x8, w8a=w8a, w8b=w8b, w8v=w8v, w8o=w8o,
                        id8=id8, bof=bof))
    return ins


_NC_CACHE = []

try:
    import jax as _jax

    _jax.clear_caches()
except Exception:
    pass


def get_nc():
    if not _NC_CACHE:
        _NC_CACHE.append(build_nc())
    return _NC_CACHE[0]


def run(in_maps, **kwargs):
    return run_bass_kernel_spmd(get_nc(), in_maps, core_ids=list(range(B)), **kwargs)


def kernel(x, Wqkv, Wo, bo):
    in_maps = host_inputs(x, Wqkv, Wo, bo)
    r = run(in_maps)
    y = np.stack([r.results[b]["y"].reshape(C, H, W) for b in range(B)])
    return y.astype(np.float32)


if __name__ == "__main__":
    nc = build_nc()
    print("built ok:", len(nc.inst_map), "instructions")


# revision 3
# speedup vs baseline: 1.3417x; 1.0034x over previous
"""Trainium2 Bass kernel v3 for nn_AttentionBlock (B=8, C=512, H=W=32, 8 heads).

Sharding: data-parallel over batch (core b owns image b; weights replicated).

All heavy matmuls run as fp8e4m3 DoubleRow (0.5 cycles/row, 2 stacked
K-planes per instruction); softmax exp runs on ACT with fp8 output and a
constant -1 logit shift (softmax-invariant) so exp() fits e4m3 range.

Per-core pipeline (x viewed as (C=512, S=1024)):
  P1qk: W-piece DR matmuls emit q,k d-SPLIT: q8/k8 (64p, 2, S) fp8 where
        partition p<32 is head A, 32<=p<64 head B, plane i = d-half.  This
        costs 2x the minimal P1 instruction count but enables...
  P2  : scores DR: lhsT=k8 (K=32, planes=d-halves) -> scoresT (128t, S) psum,
        at 0.5 cyc/row with no repacking.
  exp : ACT exp(0.125*sc - 1) -> et8 fp8 tiles (128t, 2, S); the 2-plane
        j-PAIR layout feeds attn@v DR directly.
  P3  : attn@v TALL: out(s-block 128, 65) = et8^T @ [v|1]: the 65-col output
        orientation costs out-free=65 per instruction and lands the softmax
        denominator as a per-partition COLUMN (col 64).
  norm: DVE reciprocal (free=4) + stride-0-broadcast tensor_tensor multiply
        -> res8T (s-part) fp8; PE fp8 transposes (via identity) flip it back
        to channel-partitions for...
  P4  : output projection DR + bias + residual (DVE stt) -> DMA out.
"""

import os
import sys

for _p in ("/opt/trn_rl_repo", "/root/.axon_site/_ro/trn_rl_repo"):
    if os.path.isdir(_p) and _p not in sys.path:
        sys.path.insert(0, _p)

from contextlib import ExitStack

import numpy as np
import ml_dtypes

import concourse.bass as bass
import concourse.tile as tile
from concourse import mybir
from concourse.bass_utils import run_bass_kernel_spmd

B, C, H, W = 8, 512, 32, 32
NH, D = 8, 64
S = H * W            # 1024
P = 128
NPAIR = NH // 2      # 4
NT = 8               # t-tiles (128 each)
NJJ = 4              # j-pairs
NSB = 8              # s-blocks for tall attn@v
DA = D + 1           # 65 = v cols + ones col
OQK, OV, OWO = 0, 1024, 1536   # w8 column sections
OTOT = 2048
F32 = mybir.dt.float32
F8 = mybir.dt.float8e4
AF = mybir.ActivationFunctionType
ALU = mybir.AluOpType
PM = mybir.MatmulPerfMode
EXP_SHIFT = -1.0
EXP_SCALE = 1.0 / np.sqrt(D)
DEBUG_DUMP = os.environ.get("K3_DEBUG", "0") == "1"


def _install_drain_split():
    """walrus's CTRL_NO (drain) codegen accepts only a single semaphore wait,
    but Tile's kernel-tail drain aggregates one wait per live proc.  Split
    them across several serial drains."""
    if getattr(tile.TileContext, "_drain_split_installed", False):
        return
    from concourse.vector_clock import ScopedClock

    orig = tile.TileContext._drain_and_barrier

    def patched(self, tick_clock, wait_clock):
        nc = self.nc
        drain_inst = nc.sync.drain()
        wait_clock.add_sem_waits(
            drain_inst.ins, ScopedClock({None: tick_clock.global_clock})
        )
        si = drain_inst.ins.sync_info
        if si is not None and si.on_wait and len(si.on_wait) > 1:
            waits = list(si.on_wait)
            drain_inst.ins.sync_info = mybir.SyncInfo(
                on_wait=[waits[0]], on_update=list(si.on_update or [])
            )
            for w in waits[1:]:
                d2 = nc.sync.drain()
                d2.ins.sync_info = mybir.SyncInfo(on_wait=[w], on_update=[])

        nc.all_engine_barrier()
        assert self.sems is not None
        popped = nc._tile_sem_poison_stack.pop()
        assert popped is self._sem_poison
        nc.clear_and_free_semaphores(list(self.sems.allocated().values()))
        nc.all_engine_barrier()

    tile.TileContext._drain_and_barrier = patched
    tile.TileContext._drain_split_installed = True
    tile.TileContext._drain_and_barrier_orig = orig


ENGINE_SEM_PREFIX = {
    "PE": "PE_",
    "Activation": "Activation_",
    "DVE": "DVE_",
    "Pool": "Pool_",
    "SP": "SP_",
}


def _strip_self_waits(nc):
    """Drop same-engine semaphore self-waits from multi-wait instructions
    (engines complete their own instructions in program order)."""
    n = 0
    for inst in nc.inst_map.values():
        si = getattr(inst, "sync_info", None)
        if si is None or not si.on_wait or len(si.on_wait) <= 1:
            continue
        eng = str(getattr(inst, "engine", "")).split(".")[-1]
        pref = ENGINE_SEM_PREFIX.get(eng)
        if pref is None:
            continue
        keep = [w for w in si.on_wait if not w.ant_name.startswith(pref)]
        if len(keep) != len(si.on_wait) and keep:
            inst.sync_info = mybir.SyncInfo(
                on_wait=keep, on_update=list(si.on_update or [])
            )
            n += 1
    return n


def trace_kernel(ctx, tc, nc, x, x8d, w8d, id8d, bofd, y, dbg=None):
    cst = ctx.enter_context(tc.tile_pool(name="cst", bufs=1))
    qkp = ctx.enter_context(tc.tile_pool(name="qkp", bufs=4))
    expp = ctx.enter_context(tc.tile_pool(name="expp", bufs=16))
    rstp = ctx.enter_context(tc.tile_pool(name="rstp", bufs=2))
    rdp = ctx.enter_context(tc.tile_pool(name="rdp", bufs=2))
    yp = ctx.enter_context(tc.tile_pool(name="yp", bufs=1))
    rot = ctx.enter_context(tc.tile_pool(name="rot", bufs=2, space="PSUM"))
    spp = ctx.enter_context(tc.tile_pool(name="spp", bufs=1, space="PSUM"))
    otp = ctx.enter_context(tc.tile_pool(name="otp", bufs=1, space="PSUM"))

    xt = cst.tile([P, 4, S], F32)
    x8 = cst.tile([P, 2, 2, S], F8)
    w8a = cst.tile([P, 2, 2, 256], F8)
    w8b = cst.tile([P, 2, 2, 768], F8)
    w8v = cst.tile([P, 2, 2, 512], F8)
    w8o = cst.tile([P, 2, 2, 512], F8)
    id8 = cst.tile([P, P], F8)
    bof = cst.tile([P, 5], F32)
    v8a = cst.tile([P, NJJ, 2, NH * DA], F8)
    res8 = cst.tile([P, 2, 2, S], F8)
    scr = cst.tile([1, 256], F32)
    scr8 = cst.tile([1, 64], F32)
    ybig = yp.tile([P, 4, S], F32)

    w8a_d, w8b_d, w8v_d, w8o_d = w8d
    nc.gpsimd.dma_start(out=w8a[:, :, :, :], in_=w8a_d)
    nc.sync.dma_start(out=x8[:, :, :, :], in_=x8d)
    nc.gpsimd.dma_start(out=bof[:, :], in_=bofd)
    nc.gpsimd.dma_start(out=w8v[:, :, :, :], in_=w8v_d)
    nc.sync.dma_start(out=w8b[:, :, :, :], in_=w8b_d)
    nc.gpsimd.dma_start(out=w8o[:, :, :, :], in_=w8o_d)
    nc.gpsimd.dma_start(out=id8[:, :], in_=id8d)
    nc.sync.dma_start(out=xt[:, :, :], in_=x.rearrange("(k p) s -> p k s", p=P))

    scr_i = [0]

    def dve_sync(*aps):
        # DVE wait-carrier: absorb one cross-engine wait per tiny copy.
        for ap in aps:
            n = min(ap.free_size(), 8)
            o = (scr_i[0] % 30) * 8
            scr_i[0] += 1
            nc.vector.tensor_copy(scr[0:1, o:o + n], ap)

    def pe_mm(corner, dep):
        # PE wait-carrier: a 1x2 matmul reading `dep` absorbs one cross-
        # engine wait; PE program order subsumes the tick for later matmuls.
        nc.tensor.matmul(
            corner, dep[:, 0:1], dep[:, 0:2],
            start=True, stop=True, skip_group_check=True,
        )

    # ones columns of v8a (fp8 1.0 via exp(0*x)), one ACT instr per jj;
    # reading bof also absorbs its DMA tick into the ACT clock (the exp
    # instrs' bias operand then needs no extra wait)
    # tiny ACT read of bof first: absorbs its DMA tick into the ACT clock
    dum = cst.tile([P, 1], F32)
    nc.scalar.activation(dum[:, :], bof[:, 4:5], AF.Exp, scale=0.0)
    for jj in range(NJJ):
        nc.scalar.activation(
            v8a.rearrange("p j i (h e) -> p j i h e", h=NH)[:, jj, :, :, D:DA],
            x8[:, 0, :, jj * NH:(jj + 1) * NH][:, :, :, None],
            AF.Exp, scale=0.0,
        )

    # ---------------- P1: v projection (tall: t on partitions) -------------
    # v8a[p, jj, i, h*65+e] ; v-acc psum (128t, 512 vchans)
    def p1v_jtile(j):
        acc = otp.tile([P, 512], F32, tag="ot", name=f"vacc{j}")
        if j == 0:
            pe_mm(acc[0:1, 0:2], w8v[0:1, 0, 0, 0:2])
        for kk in range(2):
            nc.tensor.matmul(
                acc[:, :],
                x8[:, kk, :, j * P:(j + 1) * P],
                w8v[:, kk, :, :],
                start=(kk == 0), stop=(kk == 1),
                perf_mode=PM.DoubleRow,
            )
        nc.vector.tensor_copy(
            v8a[:, j // 2, j % 2, :].rearrange("p (h e) -> p h e", h=NH)[:, :, 0:D],
            acc.rearrange("p (h d) -> p h d", h=NH),
        )

    # ---------------- P1: q/k pieces (d-split, M=64) ------------------------
    qk_tiles = {}

    def p1qk_piece(pair, tqk, half):
        """One M=64 piece: rows = [head 2pair (32) | head 2pair+1 (32)] of
        q-half or k-half `half`; accumulates 2 DR k-steps; evicts to fp8."""
        key = (pair, tqk)
        if key not in qk_tiles:
            qk_tiles[key] = qkp.tile(
                [D, 2, S], F8, tag="qk", name=f"qk{pair}_{tqk}")
        t8 = qk_tiles[key]
        wt = w8a if pair == 0 else w8b
        off = (pair - (0 if pair == 0 else 1)) * 256 \
            + (0 if tqk == 0 else 128) + half * D
        acc = spp.tile([D, S], F32, tag="sp", name=f"qkacc{pair}_{tqk}_{half}")
        if pair == 0 and tqk == 0 and half == 0:
            pe_mm(acc[0:1, 0:2], w8a[0:1, 0, 0, 0:2])
            pe_mm(acc[0:1, 0:2], x8[0:1, 0, 0, 0:2])
        if pair == 1 and tqk == 0 and half == 0:
            pe_mm(acc[0:1, 0:2], w8b[0:1, 0, 0, 0:2])
        for n in range(2):
            for kk in range(2):
                nc.tensor.matmul(
                    acc[:, n * 512:(n + 1) * 512],
                    wt[:, kk, :, off:off + D],
                    x8[:, kk, :, n * 512:(n + 1) * 512],
                    start=(kk == 0), stop=(kk == 1),
                    perf_mode=PM.DoubleRow,
                )
        nc.vector.tensor_copy(t8[:, half, :], acc[:, :])

    # ---------------- attention pair ---------------------------------------
    # Scores+exp stream h2-outer (all head-A j's, then head-B); the pair's
    # attn@v GROUPS (one (128,65) psum bank per (h2,sb), 4 DR jj-steps +
    # DVE recip + DVE normalize-mul) are deferred until all its exp tiles
    # exist and run as spare work inside the NEXT pair's slots.  PSUM
    # accumulation groups zero their whole 2KB bank on start, so concurrent
    # groups must own a bank: the 16 groups ping-pong through 2 banks.
    def fused_pair(pair, spare, self_h0=False):
        q8 = qk_tiles[(pair, 0)]
        k8 = qk_tiles[(pair, 1)]
        ets = {}
        slot = [0]

        def run_spare():
            s = slot[0]
            slot[0] += 1
            if s < len(spare):
                for thunk in spare[s]:
                    thunk()
            if self_h0 and 8 <= s < 16:
                k = s - 8
                group(0, k, first=(k == 0))

        res8T = rstp.tile([P, NSB, 2, D], F8, tag="rt", name=f"r8t{pair}")
        rd = rdp.tile([P, 16], F32, tag="rd", name=f"rd{pair}")

        def group(h2, sb, first=False):
            h = 2 * pair + h2
            g = h2 * NSB + sb
            ot = otp.tile([P, DA], F32, tag="ot", name=f"ot{pair}_{g}")
            if first:
                # absorb the ACT tick of this pair's last relevant exp
                pe_mm(ot[0:1, 0:2], ets[(NJJ - 1, h2)][0:1, 1, 0:2])
            for jj in range(NJJ):
                nc.tensor.matmul(
                    ot[:, :],
                    ets[(jj, h2)][:, :, sb * P:(sb + 1) * P],
                    v8a[:, jj, :, h * DA:(h + 1) * DA],
                    start=(jj == 0), stop=(jj == NJJ - 1),
                    perf_mode=PM.DoubleRow,
                    skip_group_check=True,
                )
            nc.vector.reciprocal(rd[:, g:g + 1], ot[:, D:DA])
            rdb = rd[:, g:g + 1]
            rdb = bass.AP(rdb.tensor, rdb.offset, rdb.ap[:-1] + [[0, D]])
            nc.vector.tensor_tensor(
                out=res8T[:, sb, h2, :], in0=ot[:, 0:D], in1=rdb,
                op=ALU.mult)

        def transp():
            # transposes: (128s,128c)->psum fp8 (step 2); evict to res8
            tp = spp.tile([P, 2 * S], F8, tag="sp", name=f"tp{pair}")
            tpv = tp.rearrange("p (n two) -> p n two", two=2)[:, :, 0]
            for sb in range(NSB):
                nc.tensor.matmul(
                    tpv[:, sb * P:(sb + 1) * P],
                    res8T[:, sb, :, :], id8[:, :],
                    start=True, stop=True, is_transpose=True,
                    skip_group_check=True,
                )
            nc.vector.tensor_copy(res8[:, pair // 2, pair % 2, :], tpv[:, :])

        for h2 in range(2):
            for j in range(NT):
                jj, jhalf = j // 2, j % 2
                if jhalf == 0:
                    ets[(jj, h2)] = expp.tile(
                        [P, 2, S], F8, tag="et", name=f"et{pair}_{jj}_{h2}")
                sc = rot.tile([P, S], F32, tag="sc", name=f"sc{pair}_{j}_{h2}")
                if j == 0 and h2 == 0:
                    # absorb DVE tick of this pair's q8/k8 evicts
                    pe_mm(sc[0:1, 0:2], k8[0:1, 1, 0:2])
                for n in range(2):
                    nc.tensor.matmul(
                        sc[:, n * 512:(n + 1) * 512],
                        k8[32 * h2:32 * h2 + 32, :, j * P:(j + 1) * P],
                        q8[32 * h2:32 * h2 + 32, :, n * 512:(n + 1) * 512],
                        start=True, stop=True,
                        perf_mode=PM.DoubleRow,
                    )
                nc.scalar.activation(
                    ets[(jj, h2)][:, jhalf, :], sc[:, :], AF.Exp,
                    bias=bof[:, 4:5], scale=EXP_SCALE,
                )
                run_spare()

        out = []
        h2s = [1] if self_h0 else [0, 1]
        for h2 in h2s:
            for sb in range(NSB):
                out.append(lambda h2=h2, sb=sb,
                           f=(h2 == h2s[0] and sb == 0 and not self_h0):
                           group(h2, sb, first=f))
        out.append(transp)
        return out

    # ---------------- schedule ---------------------------------------------
    # prologue: pair-0 q/k pieces + first v tiles
    p1qk_piece(0, 0, 0)
    p1qk_piece(0, 0, 1)
    p1qk_piece(0, 1, 0)
    p1qk_piece(0, 1, 1)

    def mk_slots(n):
        return [[] for _ in range(n)]

    # pair 0 spare: v tiles + pair-1 pieces
    sp0 = mk_slots(16)
    for j in range(NT):
        sp0[j].append(lambda j=j: p1v_jtile(j))
    for i in range(4):
        sp0[8 + i].append(lambda i=i: p1qk_piece(1, i // 2, i % 2))

    def id8_warm():
        # dummy transpose: absorbs the id8 DMA tick into the PE clock so
        # real transposes carry only their single DVE wait
        td = otp.tile([P, 2 * P], F8, tag="ot", name="tdum")
        tdv = td.rearrange("p (n two) -> p n two", two=2)[:, :, 0]
        nc.tensor.matmul(tdv, id8[:, :], id8[:, :],
                         start=True, stop=True, is_transpose=True,
                         skip_group_check=True)
    sp0[12].append(id8_warm)
    fin0 = fused_pair(0, sp0)

    # pair 1: pair-0 finish work (16 groups + transp) + pair-2 pieces
    sp1 = mk_slots(16)
    for k in range(16):
        sp1[k].append(fin0[k])
    sp1[15].append(fin0[16])
    for i in range(4):
        sp1[2 + 3 * (i % 2) + (i // 2)].append(
            lambda i=i: p1qk_piece(2, i // 2, i % 2))
    fin1 = fused_pair(1, sp1)

    sp2 = mk_slots(16)
    for k in range(16):
        sp2[k].append(fin1[k])
    sp2[15].append(fin1[16])
    for i in range(4):
        sp2[2 + 3 * (i % 2) + (i // 2)].append(
            lambda i=i: p1qk_piece(3, i // 2, i % 2))
    fin2 = fused_pair(2, sp2)

    # pair 3: pair-2 finish in slots 0..7 (2/slot); pair-3 head-A groups
    # self-schedule into slots 8..15 once its h0 exps are done
    sp3 = mk_slots(16)
    for k in range(16):
        sp3[k // 2].append(fin2[k])
    sp3[7].append(fin2[16])
    fin3 = fused_pair(3, sp3, self_h0=True)
    # tail: pair-3 head-B groups + its transposes
    for thunk in fin3:
        thunk()
    if dbg is not None:
        nc.sync.dma_start(out=dbg["q8"], in_=qk_tiles[(0, 0)][:, :, :])
        nc.sync.dma_start(out=dbg["k8"], in_=qk_tiles[(0, 1)][:, :, :])
        nc.sync.dma_start(out=dbg["v8a"], in_=v8a[:, :, :, :])
        nc.sync.dma_start(out=dbg["res8"], in_=res8[:, :, :, :])

    # ---------------- P4 + bias + residual + DMA out ------------------------
    # absorb xt/bof DMA ticks into the DVE clock (plain copies tolerate
    # multi-waits; TensorScalarPtr does not)
    dve_sync(xt[0:1, 0, 0:8], bof[0:1, 0:4])
    for m in range(4):
        acc = rot.tile([P, S], F32, tag="sc", name=f"p4acc{m}")
        if m == 0:
            pe_mm(acc[0:1, 0:2], res8[0:1, 1, 1, 0:2])
            pe_mm(acc[0:1, 0:2], w8o[0:1, 0, 0, 0:2])
        for n in range(2):
            for kk in range(2):
                nc.tensor.matmul(
                    acc[:, n * 512:(n + 1) * 512],
                    w8o[:, kk, :, m * P:(m + 1) * P],
                    res8[:, kk, :, n * 512:(n + 1) * 512],
                    start=(kk == 0), stop=(kk == 1),
                    perf_mode=PM.DoubleRow,
                )
        nc.vector.scalar_tensor_tensor(
            ybig[:, m, :], acc[:, :], bof[:, m:m + 1],
            xt[:, m, :], op0=ALU.add, op1=ALU.add,
        )
        yr = y.rearrange("(k p) s -> p k s", p=P)
        if m % 2 == 0:
            nc.gpsimd.tensor_copy(scr8[0:1, m * 8:m * 8 + 8], ybig[0:1, m, 0:8])
            nc.gpsimd.dma_start(out=yr[:, m:m + 1, :], in_=ybig[:, m:m + 1, :])
        else:
            nc.sync.dma_start(out=yr[:, m:m + 1, :], in_=ybig[:, m:m + 1, :])


def build_nc():
    _install_drain_split()
    nc = bass.Bass(trn_type="TRN2", debug=False, num_devices=8)
    x_d = nc.dram_tensor("x", [C, S], F32, kind="ExternalInput")
    x8_d = nc.dram_tensor("x8", [P, 2, 2, S], F8, kind="ExternalInput")
    w8a_d = nc.dram_tensor("w8a", [P, 2, 2, 256], F8, kind="ExternalInput")
    w8b_d = nc.dram_tensor("w8b", [P, 2, 2, 768], F8, kind="ExternalInput")
    w8v_d = nc.dram_tensor("w8v", [P, 2, 2, 512], F8, kind="ExternalInput")
    w8o_d = nc.dram_tensor("w8o", [P, 2, 2, 512], F8, kind="ExternalInput")
    id8_d = nc.dram_tensor("id8", [P, P], F8, kind="ExternalInput")
    bof_d = nc.dram_tensor("bof", [P, 5], F32, kind="ExternalInput")
    y_d = nc.dram_tensor("y", [C, S], F32, kind="ExternalOutput")
    dbg = None
    if DEBUG_DUMP:
        dbg = {
            "q8": nc.dram_tensor("dbg_q8", [D, 2, S], F8, kind="ExternalOutput").ap(),
            "k8": nc.dram_tensor("dbg_k8", [D, 2, S], F8, kind="ExternalOutput").ap(),
            "v8a": nc.dram_tensor("dbg_v8a", [P, NJJ, 2, NH * DA], F8, kind="ExternalOutput").ap(),
            "res8": nc.dram_tensor("dbg_res8", [P, 2, 2, S], F8, kind="ExternalOutput").ap(),
        }
    with tile.TileContext(nc) as tc, ExitStack() as ctx:
        trace_kernel(ctx, tc, nc, x_d.ap(), x8_d.ap(),
                     (w8a_d.ap(), w8b_d.ap(), w8v_d.ap(), w8o_d.ap()),
                     id8_d.ap(), bof_d.ap(), y_d.ap(), dbg)
    _strip_self_waits(nc)
    if not nc.is_finalized():
        nc.finalize()
    return nc


def host_inputs(x, Wqkv, Wo, bo):
    """Host-side reshard + fp8 quantization (weights replicated)."""
    f8 = ml_dtypes.float8_e4m3fn
    x = np.ascontiguousarray(np.asarray(x, dtype=np.float32))
    Wqkv = np.asarray(Wqkv, dtype=np.float32)
    Wo = np.asarray(Wo, dtype=np.float32)
    bo = np.asarray(bo, dtype=np.float32)

    # Wqkv rows per head h: [h*192, +64) = q, [+64, +128) = k, [+128, +192) = v
    wbig = np.empty((OTOT, C), dtype=np.float32)
    for pair in range(NPAIR):
        hA, hB = 2 * pair, 2 * pair + 1
        base = pair * 256
        for tqk, roff in ((0, 0), (1, D)):          # q rows, then k rows
            for half in range(2):
                o = base + tqk * 128 + half * D
                rA = hA * 192 + roff + half * 32
                rB = hB * 192 + roff + half * 32
                wbig[o:o + 32] = Wqkv[rA:rA + 32]
                wbig[o + 32:o + 64] = Wqkv[rB:rB + 32]
    for h in range(NH):
        wbig[OV + h * D:OV + (h + 1) * D] = Wqkv[h * 192 + 128:h * 192 + 192]
    wbig[OWO:OWO + C] = Wo
    # w8[p, kk, i, o] = wbig[o, (2kk+i)*128+p], shipped as 4 contiguous
    # section tensors so each DMA sprays across all engines
    w8 = wbig.T.reshape(2, 2, P, OTOT).transpose(2, 0, 1, 3).astype(f8)
    w8a = np.ascontiguousarray(w8[:, :, :, 0:256])
    w8b = np.ascontiguousarray(w8[:, :, :, 256:OV])
    w8v = np.ascontiguousarray(w8[:, :, :, OV:OV + 512])
    w8o = np.ascontiguousarray(w8[:, :, :, OWO:OTOT])
    id8 = np.eye(P, dtype=np.float32).astype(f8)
    bof = np.concatenate(
        [bo.reshape(4, P).T, np.full((P, 1), EXP_SHIFT, np.float32)], axis=1)
    bof = np.ascontiguousarray(bof)

    ins = []
    for b in range(B):
        xs = np.ascontiguousarray(x[b].reshape(C, S))
        x8 = np.ascontiguousarray(
            xs.reshape(2, 2, P, S).transpose(2, 0, 1, 3)).astype(f8)
        ins.append(dict(x=xs, x8=x8, w8a=w8a, w8b=w8b, w8v=w8v, w8o=w8o,
                        id8=id8, bof=bof))
    return ins


_NC_CACHE = []

try:
    import jax as _jax

    _jax.clear_caches()
except Exception:
    pass


def get_nc():
    if not _NC_CACHE:
        _NC_CACHE.append(build_nc())
    return _NC_CACHE[0]


def run(in_maps, **kwargs):
    return run_bass_kernel_spmd(get_nc(), in_maps, core_ids=list(range(B)), **kwargs)


def kernel(x, Wqkv, Wo, bo):
    in_maps = host_inputs(x, Wqkv, Wo, bo)
    r = run(in_maps)
    y = np.stack([r.results[b]["y"].reshape(C, H, W) for b in range(B)])
    return y.astype(np.float32)


if __name__ == "__main__":
    nc = build_nc()
    print("built ok:", len(nc.inst_map), "instructions")


# revision 4
# speedup vs baseline: 1.3883x; 1.0347x over previous
"""Trainium2 Bass kernel v3 for nn_AttentionBlock (B=8, C=512, H=W=32, 8 heads).

Sharding: data-parallel over batch (core b owns image b; weights replicated).

All heavy matmuls run as fp8e4m3 DoubleRow (0.5 cycles/row, 2 stacked
K-planes per instruction); softmax exp runs on ACT with fp8 output and a
constant -1 logit shift (softmax-invariant) so exp() fits e4m3 range.

Per-core pipeline (x viewed as (C=512, S=1024)):
  P1qk: W-piece DR matmuls emit q,k d-SPLIT: q8/k8 (64p, 2, S) fp8 where
        partition p<32 is head A, 32<=p<64 head B, plane i = d-half.  This
        costs 2x the minimal P1 instruction count but enables...
  P2  : scores DR: lhsT=k8 (K=32, planes=d-halves) -> scoresT (128t, S) psum,
        at 0.5 cyc/row with no repacking.
  exp : ACT exp(0.125*sc - 1) -> et8 fp8 tiles (128t, 2, S); the 2-plane
        j-PAIR layout feeds attn@v DR directly.
  P3  : attn@v TALL: out(s-block 128, 65) = et8^T @ [v|1]: the 65-col output
        orientation costs out-free=65 per instruction and lands the softmax
        denominator as a per-partition COLUMN (col 64).
  norm: DVE reciprocal (free=4) + stride-0-broadcast tensor_tensor multiply
        -> res8T (s-part) fp8; PE fp8 transposes (via identity) flip it back
        to channel-partitions for...
  P4  : output projection DR + bias + residual (DVE stt) -> DMA out.
"""

import os
import sys

for _p in ("/opt/trn_rl_repo", "/root/.axon_site/_ro/trn_rl_repo"):
    if os.path.isdir(_p) and _p not in sys.path:
        sys.path.insert(0, _p)

from contextlib import ExitStack

import numpy as np
import ml_dtypes

import concourse.bass as bass
import concourse.tile as tile
from concourse import mybir
from concourse.bass_utils import run_bass_kernel_spmd

B, C, H, W = 8, 512, 32, 32
NH, D = 8, 64
S = H * W            # 1024
P = 128
NPAIR = NH // 2      # 4
NT = 8               # t-tiles (128 each)
NJJ = 4              # j-pairs
NSB = 8              # s-blocks for tall attn@v
DA = D + 1           # 65 = v cols + ones col
OQK, OV, OWO = 0, 1024, 1536   # w8 column sections
OTOT = 2048
F32 = mybir.dt.float32
F8 = mybir.dt.float8e4
AF = mybir.ActivationFunctionType
ALU = mybir.AluOpType
PM = mybir.MatmulPerfMode
EXP_SHIFT = -1.0
EXP_SCALE = 1.0 / np.sqrt(D)
DEBUG_DUMP = os.environ.get("K3_DEBUG", "0") == "1"


def _install_drain_split():
    """walrus's CTRL_NO (drain) codegen accepts only a single semaphore wait,
    but Tile's kernel-tail drain aggregates one wait per live proc.  Split
    them across several serial drains."""
    if getattr(tile.TileContext, "_drain_split_installed", False):
        return
    from concourse.vector_clock import ScopedClock

    orig = tile.TileContext._drain_and_barrier

    def patched(self, tick_clock, wait_clock):
        nc = self.nc
        drain_inst = nc.sync.drain()
        wait_clock.add_sem_waits(
            drain_inst.ins, ScopedClock({None: tick_clock.global_clock})
        )
        si = drain_inst.ins.sync_info
        if si is not None and si.on_wait and len(si.on_wait) > 1:
            waits = list(si.on_wait)
            drain_inst.ins.sync_info = mybir.SyncInfo(
                on_wait=[waits[0]], on_update=list(si.on_update or [])
            )
            for w in waits[1:]:
                d2 = nc.sync.drain()
                d2.ins.sync_info = mybir.SyncInfo(on_wait=[w], on_update=[])

        nc.all_engine_barrier()
        assert self.sems is not None
        popped = nc._tile_sem_poison_stack.pop()
        assert popped is self._sem_poison
        nc.clear_and_free_semaphores(list(self.sems.allocated().values()))
        nc.all_engine_barrier()

    tile.TileContext._drain_and_barrier = patched
    tile.TileContext._drain_split_installed = True
    tile.TileContext._drain_and_barrier_orig = orig


ENGINE_SEM_PREFIX = {
    "PE": "PE_",
    "Activation": "Activation_",
    "DVE": "DVE_",
    "Pool": "Pool_",
    "SP": "SP_",
}


def _strip_self_waits(nc):
    """Drop same-engine semaphore self-waits from multi-wait instructions
    (engines complete their own instructions in program order)."""
    n = 0
    for inst in nc.inst_map.values():
        si = getattr(inst, "sync_info", None)
        if si is None or not si.on_wait or len(si.on_wait) <= 1:
            continue
        eng = str(getattr(inst, "engine", "")).split(".")[-1]
        pref = ENGINE_SEM_PREFIX.get(eng)
        if pref is None:
            continue
        keep = [w for w in si.on_wait if not w.ant_name.startswith(pref)]
        if len(keep) != len(si.on_wait) and keep:
            inst.sync_info = mybir.SyncInfo(
                on_wait=keep, on_update=list(si.on_update or [])
            )
            n += 1
    return n


def trace_kernel(ctx, tc, nc, x, x8d, w8d, id8d, bofd, y, dbg=None):
    cst = ctx.enter_context(tc.tile_pool(name="cst", bufs=1))
    qkp = ctx.enter_context(tc.tile_pool(name="qkp", bufs=4))
    expp = ctx.enter_context(tc.tile_pool(name="expp", bufs=16))
    rstp = ctx.enter_context(tc.tile_pool(name="rstp", bufs=2))
    rdp = ctx.enter_context(tc.tile_pool(name="rdp", bufs=2))
    yp = ctx.enter_context(tc.tile_pool(name="yp", bufs=1))
    rot = ctx.enter_context(tc.tile_pool(name="rot", bufs=3, space="PSUM"))
    spp = ctx.enter_context(tc.tile_pool(name="spp", bufs=1, space="PSUM"))
    otp = ctx.enter_context(tc.tile_pool(name="otp", bufs=1, space="PSUM"))

    xt = cst.tile([P, 4, S], F32)
    x8 = cst.tile([P, 2, 2, S], F8)
    w8a = cst.tile([P, 2, 2, 256], F8)
    w8b = cst.tile([P, 2, 2, 768], F8)
    w8v = cst.tile([P, 2, 2, 512], F8)
    w8o = cst.tile([P, 2, 2, 512], F8)
    id8 = cst.tile([P, P], F8)
    bof = cst.tile([P, 5], F32)
    v8a = cst.tile([P, NJJ, 2, NH * DA], F8)
    res8 = cst.tile([P, 2, 2, S], F8)
    scr = cst.tile([1, 256], F32)
    scr8 = cst.tile([1, 64], F32)
    ybig = yp.tile([P, 4, S], F32)

    w8a_d, w8b_d, w8v_d, w8o_d = w8d
    nc.gpsimd.dma_start(out=w8a[:, :, :, :], in_=w8a_d)
    nc.sync.dma_start(out=x8[:, :, :, :], in_=x8d)
    nc.gpsimd.dma_start(out=bof[:, :], in_=bofd)
    nc.gpsimd.dma_start(out=w8v[:, :, :, :], in_=w8v_d)
    nc.sync.dma_start(out=w8b[:, :, :, :], in_=w8b_d)
    nc.gpsimd.dma_start(out=w8o[:, :, :, :], in_=w8o_d)
    nc.gpsimd.dma_start(out=id8[:, :], in_=id8d)
    nc.sync.dma_start(out=xt[:, :, :], in_=x.rearrange("(k p) s -> p k s", p=P))

    scr_i = [0]

    def dve_sync(*aps):
        # DVE wait-carrier: absorb one cross-engine wait per tiny copy.
        for ap in aps:
            n = min(ap.free_size(), 8)
            o = (scr_i[0] % 30) * 8
            scr_i[0] += 1
            nc.vector.tensor_copy(scr[0:1, o:o + n], ap)

    def pe_mm(corner, dep):
        # PE wait-carrier: a 1x2 matmul reading `dep` absorbs one cross-
        # engine wait; PE program order subsumes the tick for later matmuls.
        nc.tensor.matmul(
            corner, dep[:, 0:1], dep[:, 0:2],
            start=True, stop=True, skip_group_check=True,
        )

    # ones columns of v8a (fp8 1.0 via exp(0*x)), one ACT instr per jj;
    # reading bof also absorbs its DMA tick into the ACT clock (the exp
    # instrs' bias operand then needs no extra wait)
    # tiny ACT read of bof first: absorbs its DMA tick into the ACT clock
    dum = cst.tile([P, 1], F32)
    nc.scalar.activation(dum[:, :], bof[:, 4:5], AF.Exp, scale=0.0)
    for jj in range(NJJ):
        nc.scalar.activation(
            v8a.rearrange("p j i (h e) -> p j i h e", h=NH)[:, jj, :, :, D:DA],
            x8[:, 0, :, jj * NH:(jj + 1) * NH][:, :, :, None],
            AF.Exp, scale=0.0,
        )

    # ---------------- P1: v projection (tall: t on partitions) -------------
    # v8a[p, jj, i, h*65+e] ; v-acc psum (128t, 512 vchans)
    def p1v_jtile(j):
        acc = rot.tile([P, 512], F32, tag="sc", name=f"vacc{j}")
        if j == 0:
            pe_mm(acc[0:1, 0:2], w8v[0:1, 0, 0, 0:2])
        for kk in range(2):
            nc.tensor.matmul(
                acc[:, :],
                x8[:, kk, :, j * P:(j + 1) * P],
                w8v[:, kk, :, :],
                start=(kk == 0), stop=(kk == 1),
                perf_mode=PM.DoubleRow,
            )
        nc.vector.tensor_copy(
            v8a[:, j // 2, j % 2, :].rearrange("p (h e) -> p h e", h=NH)[:, :, 0:D],
            acc.rearrange("p (h d) -> p h d", h=NH),
        )

    # ---------------- P1: q/k pieces (d-split, M=64) ------------------------
    qk_tiles = {}

    def p1qk_piece(pair, tqk, half):
        """One M=64 piece: rows = [head 2pair (32) | head 2pair+1 (32)] of
        q-half or k-half `half`; accumulates 2 DR k-steps; evicts to fp8."""
        key = (pair, tqk)
        if key not in qk_tiles:
            qk_tiles[key] = qkp.tile(
                [D, 2, S], F8, tag="qk", name=f"qk{pair}_{tqk}")
        t8 = qk_tiles[key]
        wt = w8a if pair == 0 else w8b
        off = (pair - (0 if pair == 0 else 1)) * 256 \
            + (0 if tqk == 0 else 128) + half * D
        for n in range(2):
            acc = spp.tile([D, 512], F32, tag="sp",
                           name=f"qkacc{pair}_{tqk}_{half}_{n}")
            if pair == 0 and tqk == 0 and half == 0 and n == 0:
                pe_mm(acc[0:1, 0:2], w8a[0:1, 0, 0, 0:2])
                pe_mm(acc[0:1, 0:2], x8[0:1, 0, 0, 0:2])
            if pair == 1 and tqk == 0 and half == 0 and n == 0:
                pe_mm(acc[0:1, 0:2], w8b[0:1, 0, 0, 0:2])
            for kk in range(2):
                nc.tensor.matmul(
                    acc[:, :],
                    wt[:, kk, :, off:off + D],
                    x8[:, kk, :, n * 512:(n + 1) * 512],
                    start=(kk == 0), stop=(kk == 1),
                    perf_mode=PM.DoubleRow,
                )
            nc.vector.tensor_copy(
                t8[:, half, n * 512:(n + 1) * 512], acc[:, :])

    # ---------------- attention pair ---------------------------------------
    # Scores+exp stream h2-outer (all head-A j's, then head-B); the pair's
    # attn@v GROUPS (one (128,65) psum bank per (h2,sb), 4 DR jj-steps +
    # DVE recip + DVE normalize-mul) are deferred until all its exp tiles
    # exist and run as spare work inside the NEXT pair's slots.  PSUM
    # accumulation groups zero their whole 2KB bank on start, so concurrent
    # groups must own a bank: the 16 groups ping-pong through 2 banks.
    def fused_pair(pair, spare, self_h0=False):
        q8 = qk_tiles[(pair, 0)]
        k8 = qk_tiles[(pair, 1)]
        ets = {}
        slot = [0]

        def run_spare():
            s = slot[0]
            slot[0] += 1
            if s < len(spare):
                for thunk in spare[s]:
                    thunk()
            if self_h0 and 8 <= s < 16:
                k = s - 8
                group(0, k, first=(k == 0))

        res8T = rstp.tile([P, NSB, 2, D], F8, tag="rt", name=f"r8t{pair}")
        rd = rdp.tile([P, 16], F32, tag="rd", name=f"rd{pair}")

        def group(h2, sb, first=False):
            h = 2 * pair + h2
            g = h2 * NSB + sb
            pool, tg = (otp, "ot") if g % 2 == 0 else (spp, "sp")
            ot = pool.tile([P, DA], F32, tag=tg, name=f"ot{pair}_{g}")
            if first:
                # absorb the ACT tick of this pair's last relevant exp
                pe_mm(ot[0:1, 0:2], ets[(NJJ - 1, h2)][0:1, 1, 0:2])
            for jj in range(NJJ):
                nc.tensor.matmul(
                    ot[:, :],
                    ets[(jj, h2)][:, :, sb * P:(sb + 1) * P],
                    v8a[:, jj, :, h * DA:(h + 1) * DA],
                    start=(jj == 0), stop=(jj == NJJ - 1),
                    perf_mode=PM.DoubleRow,
                    skip_group_check=True,
                )
            nc.vector.reciprocal(rd[:, g:g + 1], ot[:, D:DA])
            rdb = rd[:, g:g + 1]
            rdb = bass.AP(rdb.tensor, rdb.offset, rdb.ap[:-1] + [[0, D]])
            nc.vector.tensor_tensor(
                out=res8T[:, sb, h2, :], in0=ot[:, 0:D], in1=rdb,
                op=ALU.mult)

        def transp():
            # transposes: (128s,128c)->psum fp8 (step 2); evict to res8
            tp = spp.tile([P, 2 * S], F8, tag="sp", name=f"tp{pair}")
            tpv = tp.rearrange("p (n two) -> p n two", two=2)[:, :, 0]
            for sb in range(NSB):
                nc.tensor.matmul(
                    tpv[:, sb * P:(sb + 1) * P],
                    res8T[:, sb, :, :], id8[:, :],
                    start=True, stop=True, is_transpose=True,
                    skip_group_check=True,
                )
            nc.vector.tensor_copy(res8[:, pair // 2, pair % 2, :], tpv[:, :])

        for h2 in range(2):
            for j in range(NT):
                jj, jhalf = j // 2, j % 2
                if jhalf == 0:
                    ets[(jj, h2)] = expp.tile(
                        [P, 2, S], F8, tag="et", name=f"et{pair}_{jj}_{h2}")
                sc = rot.tile([P, S], F32, tag="sc", name=f"sc{pair}_{j}_{h2}")
                if j == 0 and h2 == 0:
                    # absorb DVE tick of this pair's q8/k8 evicts
                    pe_mm(sc[0:1, 0:2], k8[0:1, 1, 0:2])
                for n in range(2):
                    nc.tensor.matmul(
                        sc[:, n * 512:(n + 1) * 512],
                        k8[32 * h2:32 * h2 + 32, :, j * P:(j + 1) * P],
                        q8[32 * h2:32 * h2 + 32, :, n * 512:(n + 1) * 512],
                        start=True, stop=True,
                        perf_mode=PM.DoubleRow,
                    )
                nc.scalar.activation(
                    ets[(jj, h2)][:, jhalf, :], sc[:, :], AF.Exp,
                    bias=bof[:, 4:5], scale=EXP_SCALE,
                )
                run_spare()

        out = []
        h2s = [1] if self_h0 else [0, 1]
        for h2 in h2s:
            for sb in range(NSB):
                out.append(lambda h2=h2, sb=sb,
                           f=(h2 == h2s[0] and sb == 0 and not self_h0):
                           group(h2, sb, first=f))
        out.append(transp)
        return out

    # ---------------- schedule ---------------------------------------------
    # prologue: pair-0 q/k pieces + first v tiles
    p1qk_piece(0, 0, 0)
    p1qk_piece(0, 0, 1)
    p1qk_piece(0, 1, 0)
    p1qk_piece(0, 1, 1)

    def mk_slots(n):
        return [[] for _ in range(n)]

    # pair 0 spare: v tiles + pair-1 pieces
    sp0 = mk_slots(16)
    for j in range(NT):
        sp0[j].append(lambda j=j: p1v_jtile(j))
    for i in range(4):
        sp0[8 + i].append(lambda i=i: p1qk_piece(1, i // 2, i % 2))

    def id8_warm():
        # dummy transpose: absorbs the id8 DMA tick into the PE clock so
        # real transposes carry only their single DVE wait
        td = otp.tile([P, 2 * P], F8, tag="ot", name="tdum")
        tdv = td.rearrange("p (n two) -> p n two", two=2)[:, :, 0]
        nc.tensor.matmul(tdv, id8[:, :], id8[:, :],
                         start=True, stop=True, is_transpose=True,
                         skip_group_check=True)
    sp0[12].append(id8_warm)
    fin0 = fused_pair(0, sp0)

    # pair 1: pair-0 finish work (16 groups + transp) + pair-2 pieces
    sp1 = mk_slots(16)
    for k in range(16):
        sp1[k].append(fin0[k])
    sp1[15].append(fin0[16])
    for i in range(4):
        sp1[2 + 3 * (i % 2) + (i // 2)].append(
            lambda i=i: p1qk_piece(2, i // 2, i % 2))
    fin1 = fused_pair(1, sp1)

    sp2 = mk_slots(16)
    for k in range(16):
        sp2[k].append(fin1[k])
    sp2[15].append(fin1[16])
    for i in range(4):
        sp2[2 + 3 * (i % 2) + (i // 2)].append(
            lambda i=i: p1qk_piece(3, i // 2, i % 2))
    fin2 = fused_pair(2, sp2)

    # pair 3: pair-2 finish in slots 0..7 (2/slot); pair-3 head-A groups
    # self-schedule into slots 8..15 once its h0 exps are done
    sp3 = mk_slots(16)
    for k in range(16):
        sp3[k // 2].append(fin2[k])
    sp3[7].append(fin2[16])
    fin3 = fused_pair(3, sp3, self_h0=True)
    # tail: pair-3 head-B groups + its transposes
    for thunk in fin3:
        thunk()
    if dbg is not None:
        nc.sync.dma_start(out=dbg["q8"], in_=qk_tiles[(0, 0)][:, :, :])
        nc.sync.dma_start(out=dbg["k8"], in_=qk_tiles[(0, 1)][:, :, :])
        nc.sync.dma_start(out=dbg["v8a"], in_=v8a[:, :, :, :])
        nc.sync.dma_start(out=dbg["res8"], in_=res8[:, :, :, :])

    # ---------------- P4 + bias + residual + DMA out ------------------------
    # absorb xt/bof DMA ticks into the DVE clock (plain copies tolerate
    # multi-waits; TensorScalarPtr does not)
    dve_sync(xt[0:1, 0, 0:8], bof[0:1, 0:4])
    for m in range(4):
        acc = rot.tile([P, S], F32, tag="sc", name=f"p4acc{m}")
        if m == 0:
            pe_mm(acc[0:1, 0:2], res8[0:1, 1, 1, 0:2])
            pe_mm(acc[0:1, 0:2], w8o[0:1, 0, 0, 0:2])
        for n in range(2):
            for kk in range(2):
                nc.tensor.matmul(
                    acc[:, n * 512:(n + 1) * 512],
                    w8o[:, kk, :, m * P:(m + 1) * P],
                    res8[:, kk, :, n * 512:(n + 1) * 512],
                    start=(kk == 0), stop=(kk == 1),
                    perf_mode=PM.DoubleRow,
                )
        nc.vector.scalar_tensor_tensor(
            ybig[:, m, :], acc[:, :], bof[:, m:m + 1],
            xt[:, m, :], op0=ALU.add, op1=ALU.add,
        )
        yr = y.rearrange("(k p) s -> p k s", p=P)
        if m % 2 == 0:
            nc.gpsimd.tensor_copy(scr8[0:1, m * 8:m * 8 + 8], ybig[0:1, m, 0:8])
            nc.gpsimd.dma_start(out=yr[:, m:m + 1, :], in_=ybig[:, m:m + 1, :])
        else:
            nc.sync.dma_start(out=yr[:, m:m + 1, :], in_=ybig[:, m:m + 1, :])


def build_nc():
    _install_drain_split()
    nc = bass.Bass(trn_type="TRN2", debug=False, num_devices=8)
    x_d = nc.dram_tensor("x", [C, S], F32, kind="ExternalInput")
    x8_d = nc.dram_tensor("x8", [P, 2, 2, S], F8, kind="ExternalInput")
    w8a_d = nc.dram_tensor("w8a", [P, 2, 2, 256], F8, kind="ExternalInput")
    w8b_d = nc.dram_tensor("w8b", [P, 2, 2, 768], F8, kind="ExternalInput")
    w8v_d = nc.dram_tensor("w8v", [P, 2, 2, 512], F8, kind="ExternalInput")
    w8o_d = nc.dram_tensor("w8o", [P, 2, 2, 512], F8, kind="ExternalInput")
    id8_d = nc.dram_tensor("id8", [P, P], F8, kind="ExternalInput")
    bof_d = nc.dram_tensor("bof", [P, 5], F32, kind="ExternalInput")
    y_d = nc.dram_tensor("y", [C, S], F32, kind="ExternalOutput")
    dbg = None
    if DEBUG_DUMP:
        dbg = {
            "q8": nc.dram_tensor("dbg_q8", [D, 2, S], F8, kind="ExternalOutput").ap(),
            "k8": nc.dram_tensor("dbg_k8", [D, 2, S], F8, kind="ExternalOutput").ap(),
            "v8a": nc.dram_tensor("dbg_v8a", [P, NJJ, 2, NH * DA], F8, kind="ExternalOutput").ap(),
            "res8": nc.dram_tensor("dbg_res8", [P, 2, 2, S], F8, kind="ExternalOutput").ap(),
        }
    with tile.TileContext(nc) as tc, ExitStack() as ctx:
        trace_kernel(ctx, tc, nc, x_d.ap(), x8_d.ap(),
                     (w8a_d.ap(), w8b_d.ap(), w8v_d.ap(), w8o_d.ap()),
                     id8_d.ap(), bof_d.ap(), y_d.ap(), dbg)
    _strip_self_waits(nc)
    if not nc.is_finalized():
        nc.finalize()
    return nc


def host_inputs(x, Wqkv, Wo, bo):
    """Host-side reshard + fp8 quantization (weights replicated)."""
    f8 = ml_dtypes.float8_e4m3fn
    x = np.ascontiguousarray(np.asarray(x, dtype=np.float32))
    Wqkv = np.asarray(Wqkv, dtype=np.float32)
    Wo = np.asarray(Wo, dtype=np.float32)
    bo = np.asarray(bo, dtype=np.float32)

    # Wqkv rows per head h: [h*192, +64) = q, [+64, +128) = k, [+128, +192) = v
    wbig = np.empty((OTOT, C), dtype=np.float32)
    for pair in range(NPAIR):
        hA, hB = 2 * pair, 2 * pair + 1
        base = pair * 256
        for tqk, roff in ((0, 0), (1, D)):          # q rows, then k rows
            for half in range(2):
                o = base + tqk * 128 + half * D
                rA = hA * 192 + roff + half * 32
                rB = hB * 192 + roff + half * 32
                wbig[o:o + 32] = Wqkv[rA:rA + 32]
                wbig[o + 32:o + 64] = Wqkv[rB:rB + 32]
    for h in range(NH):
        wbig[OV + h * D:OV + (h + 1) * D] = Wqkv[h * 192 + 128:h * 192 + 192]
    wbig[OWO:OWO + C] = Wo
    # w8[p, kk, i, o] = wbig[o, (2kk+i)*128+p], shipped as 4 contiguous
    # section tensors so each DMA sprays across all engines
    w8 = wbig.T.reshape(2, 2, P, OTOT).transpose(2, 0, 1, 3).astype(f8)
    w8a = np.ascontiguousarray(w8[:, :, :, 0:256])
    w8b = np.ascontiguousarray(w8[:, :, :, 256:OV])
    w8v = np.ascontiguousarray(w8[:, :, :, OV:OV + 512])
    w8o = np.ascontiguousarray(w8[:, :, :, OWO:OTOT])
    id8 = np.eye(P, dtype=np.float32).astype(f8)
    bof = np.concatenate(
        [bo.reshape(4, P).T, np.full((P, 1), EXP_SHIFT, np.float32)], axis=1)
    bof = np.ascontiguousarray(bof)

    ins = []
    for b in range(B):
        xs = np.ascontiguousarray(x[b].reshape(C, S))
        x8 = np.ascontiguousarray(
            xs.reshape(2, 2, P, S).transpose(2, 0, 1, 3)).astype(f8)
        ins.append(dict(x=xs, x8=x8, w8a=w8a, w8b=w8b, w8v=w8v, w8o=w8o,
                        id8=id8, bof=bof))
    return ins


_NC_CACHE = []

try:
    import jax as _jax

    _jax.clear_caches()
except Exception:
    pass


def get_nc():
    if not _NC_CACHE:
        _NC_CACHE.append(build_nc())
    return _NC_CACHE[0]


def run(in_maps, **kwargs):
    return run_bass_kernel_spmd(get_nc(), in_maps, core_ids=list(range(B)), **kwargs)


def kernel(x, Wqkv, Wo, bo):
    in_maps = host_inputs(x, Wqkv, Wo, bo)
    r = run(in_maps)
    y = np.stack([r.results[b]["y"].reshape(C, H, W) for b in range(B)])
    return y.astype(np.float32)


if __name__ == "__main__":
    nc = build_nc()
    print("built ok:", len(nc.inst_map), "instructions")


# revision 5
# speedup vs baseline: 1.3916x; 1.0024x over previous
"""Trainium2 Bass kernel v3 for nn_AttentionBlock (B=8, C=512, H=W=32, 8 heads).

Sharding: data-parallel over batch (core b owns image b; weights replicated).

All heavy matmuls run as fp8e4m3 DoubleRow (0.5 cycles/row, 2 stacked
K-planes per instruction); softmax exp runs on ACT with fp8 output and a
constant -1 logit shift (softmax-invariant) so exp() fits e4m3 range.

Per-core pipeline (x viewed as (C=512, S=1024)):
  P1qk: W-piece DR matmuls emit q,k d-SPLIT: q8/k8 (64p, 2, S) fp8 where
        partition p<32 is head A, 32<=p<64 head B, plane i = d-half.  This
        costs 2x the minimal P1 instruction count but enables...
  P2  : scores DR: lhsT=k8 (K=32, planes=d-halves) -> scoresT (128t, S) psum,
        at 0.5 cyc/row with no repacking.
  exp : ACT exp(0.125*sc - 1) -> et8 fp8 tiles (128t, 2, S); the 2-plane
        j-PAIR layout feeds attn@v DR directly.
  P3  : attn@v TALL: out(s-block 128, 65) = et8^T @ [v|1]: the 65-col output
        orientation costs out-free=65 per instruction and lands the softmax
        denominator as a per-partition COLUMN (col 64).
  norm: DVE reciprocal (free=4) + stride-0-broadcast tensor_tensor multiply
        -> res8T (s-part) fp8; PE fp8 transposes (via identity) flip it back
        to channel-partitions for...
  P4  : output projection DR + bias + residual (DVE stt) -> DMA out.
"""

import os
import sys

for _p in ("/opt/trn_rl_repo", "/root/.axon_site/_ro/trn_rl_repo"):
    if os.path.isdir(_p) and _p not in sys.path:
        sys.path.insert(0, _p)

from contextlib import ExitStack

import numpy as np
import ml_dtypes

import concourse.bass as bass
import concourse.tile as tile
from concourse import mybir
from concourse.bass_utils import run_bass_kernel_spmd

B, C, H, W = 8, 512, 32, 32
NH, D = 8, 64
S = H * W            # 1024
P = 128
NPAIR = NH // 2      # 4
NT = 8               # t-tiles (128 each)
NJJ = 4              # j-pairs
NSB = 8              # s-blocks for tall attn@v
DA = D + 1           # 65 = v cols + ones col
OQK, OV, OWO = 0, 1024, 1536   # w8 column sections
OTOT = 2048
F32 = mybir.dt.float32
F8 = mybir.dt.float8e4
AF = mybir.ActivationFunctionType
ALU = mybir.AluOpType
PM = mybir.MatmulPerfMode
EXP_SHIFT = -1.0
EXP_SCALE = 1.0 / np.sqrt(D)
DEBUG_DUMP = os.environ.get("K3_DEBUG", "0") == "1"


def _install_drain_split():
    """walrus's CTRL_NO (drain) codegen accepts only a single semaphore wait,
    but Tile's kernel-tail drain aggregates one wait per live proc.  Split
    them across several serial drains."""
    if getattr(tile.TileContext, "_drain_split_installed", False):
        return
    from concourse.vector_clock import ScopedClock

    orig = tile.TileContext._drain_and_barrier

    def patched(self, tick_clock, wait_clock):
        nc = self.nc
        drain_inst = nc.sync.drain()
        wait_clock.add_sem_waits(
            drain_inst.ins, ScopedClock({None: tick_clock.global_clock})
        )
        si = drain_inst.ins.sync_info
        if si is not None and si.on_wait and len(si.on_wait) > 1:
            waits = list(si.on_wait)
            drain_inst.ins.sync_info = mybir.SyncInfo(
                on_wait=[waits[0]], on_update=list(si.on_update or [])
            )
            for w in waits[1:]:
                d2 = nc.sync.drain()
                d2.ins.sync_info = mybir.SyncInfo(on_wait=[w], on_update=[])

        nc.all_engine_barrier()
        assert self.sems is not None
        popped = nc._tile_sem_poison_stack.pop()
        assert popped is self._sem_poison
        nc.clear_and_free_semaphores(list(self.sems.allocated().values()))
        nc.all_engine_barrier()

    tile.TileContext._drain_and_barrier = patched
    tile.TileContext._drain_split_installed = True
    tile.TileContext._drain_and_barrier_orig = orig


ENGINE_SEM_PREFIX = {
    "PE": "PE_",
    "Activation": "Activation_",
    "DVE": "DVE_",
    "Pool": "Pool_",
    "SP": "SP_",
}


def _strip_self_waits(nc):
    """Drop same-engine semaphore self-waits from multi-wait instructions
    (engines complete their own instructions in program order)."""
    n = 0
    for inst in nc.inst_map.values():
        si = getattr(inst, "sync_info", None)
        if si is None or not si.on_wait or len(si.on_wait) <= 1:
            continue
        eng = str(getattr(inst, "engine", "")).split(".")[-1]
        pref = ENGINE_SEM_PREFIX.get(eng)
        if pref is None:
            continue
        keep = [w for w in si.on_wait if not w.ant_name.startswith(pref)]
        if len(keep) != len(si.on_wait) and keep:
            inst.sync_info = mybir.SyncInfo(
                on_wait=keep, on_update=list(si.on_update or [])
            )
            n += 1
    return n


def trace_kernel(ctx, tc, nc, x, x8d, w8d, id8d, bofd, y, dbg=None):
    cst = ctx.enter_context(tc.tile_pool(name="cst", bufs=1))
    qkp = ctx.enter_context(tc.tile_pool(name="qkp", bufs=4))
    expp = ctx.enter_context(tc.tile_pool(name="expp", bufs=16))
    rstp = ctx.enter_context(tc.tile_pool(name="rstp", bufs=2))
    rdp = ctx.enter_context(tc.tile_pool(name="rdp", bufs=2))
    yp = ctx.enter_context(tc.tile_pool(name="yp", bufs=1))
    rot = ctx.enter_context(tc.tile_pool(name="rot", bufs=3, space="PSUM"))
    spp = ctx.enter_context(tc.tile_pool(name="spp", bufs=1, space="PSUM"))
    otp = ctx.enter_context(tc.tile_pool(name="otp", bufs=1, space="PSUM"))

    xt = cst.tile([P, 4, S], F32)
    x8 = cst.tile([P, 2, 2, S], F8)
    w8a = cst.tile([P, 2, 2, 256], F8)
    w8b = cst.tile([P, 2, 2, 768], F8)
    w8v = cst.tile([P, 2, 2, 512], F8)
    w8o = cst.tile([P, 2, 2, 512], F8)
    id8 = cst.tile([P, P], F8)
    bof = cst.tile([P, 5], F32)
    v8a = cst.tile([P, NJJ, 2, NH * DA], F8)
    res8 = cst.tile([P, 2, 2, S], F8)
    scr = cst.tile([1, 256], F32)
    scr8 = cst.tile([1, 64], F32)
    ybig = yp.tile([P, 4, S], F32)

    w8a_d, w8b_d, w8v_d, w8o_d = w8d
    nc.gpsimd.dma_start(out=w8a[:, :, :, :], in_=w8a_d)
    nc.sync.dma_start(out=x8[:, :, :, :], in_=x8d)
    nc.gpsimd.dma_start(out=bof[:, :], in_=bofd)
    nc.gpsimd.dma_start(out=w8v[:, :, :, :], in_=w8v_d)
    nc.sync.dma_start(out=w8b[:, :, :, :], in_=w8b_d)
    nc.gpsimd.dma_start(out=w8o[:, :, :, :], in_=w8o_d)
    nc.gpsimd.dma_start(out=id8[:, :], in_=id8d)
    nc.sync.dma_start(out=xt[:, :, :], in_=x.rearrange("(k p) s -> p k s", p=P))

    scr_i = [0]

    def dve_sync(*aps):
        # DVE wait-carrier: absorb one cross-engine wait per tiny copy.
        for ap in aps:
            n = min(ap.free_size(), 8)
            o = (scr_i[0] % 30) * 8
            scr_i[0] += 1
            nc.vector.tensor_copy(scr[0:1, o:o + n], ap)

    def pe_mm(corner, dep):
        # PE wait-carrier: a 1x2 matmul reading `dep` absorbs one cross-
        # engine wait; PE program order subsumes the tick for later matmuls.
        nc.tensor.matmul(
            corner, dep[:, 0:1], dep[:, 0:2],
            start=True, stop=True, skip_group_check=True,
        )

    # ones columns of v8a (fp8 1.0 via exp(0*x)), one ACT instr per jj;
    # reading bof also absorbs its DMA tick into the ACT clock (the exp
    # instrs' bias operand then needs no extra wait)
    # tiny ACT read of bof first: absorbs its DMA tick into the ACT clock
    dum = cst.tile([P, 1], F32)
    nc.scalar.activation(dum[:, :], bof[:, 4:5], AF.Exp, scale=0.0)
    for jj in range(NJJ):
        nc.scalar.activation(
            v8a.rearrange("p j i (h e) -> p j i h e", h=NH)[:, jj, :, :, D:DA],
            x8[:, 0, :, jj * NH:(jj + 1) * NH][:, :, :, None],
            AF.Exp, scale=0.0,
        )

    # ---------------- P1: v projection (tall: t on partitions) -------------
    # v8a[p, jj, i, h*65+e] ; v-acc psum (128t, 512 vchans)
    def p1v_jtile(j):
        acc = rot.tile([P, 512], F32, tag="sc", name=f"vacc{j}")
        if j == 0:
            pe_mm(acc[0:1, 0:2], w8v[0:1, 0, 0, 0:2])
        for kk in range(2):
            nc.tensor.matmul(
                acc[:, :],
                x8[:, kk, :, j * P:(j + 1) * P],
                w8v[:, kk, :, :],
                start=(kk == 0), stop=(kk == 1),
                perf_mode=PM.DoubleRow,
            )
        nc.vector.tensor_copy(
            v8a[:, j // 2, j % 2, :].rearrange("p (h e) -> p h e", h=NH)[:, :, 0:D],
            acc.rearrange("p (h d) -> p h d", h=NH),
        )

    # ---------------- P1: q/k pieces (d-split, M=64) ------------------------
    qk_tiles = {}

    def p1qk_piece(pair, tqk, half):
        """One M=64 piece: rows = [head 2pair (32) | head 2pair+1 (32)] of
        q-half or k-half `half`; accumulates 2 DR k-steps; evicts to fp8."""
        key = (pair, tqk)
        if key not in qk_tiles:
            qk_tiles[key] = qkp.tile(
                [D, 2, S], F8, tag="qk", name=f"qk{pair}_{tqk}")
        t8 = qk_tiles[key]
        wt = w8a if pair == 0 else w8b
        off = (pair - (0 if pair == 0 else 1)) * 256 \
            + (0 if tqk == 0 else 128) + half * D
        for n in range(2):
            acc = spp.tile([D, 512], F32, tag="sp",
                           name=f"qkacc{pair}_{tqk}_{half}_{n}")
            if pair == 0 and tqk == 0 and half == 0 and n == 0:
                pe_mm(acc[0:1, 0:2], w8a[0:1, 0, 0, 0:2])
                pe_mm(acc[0:1, 0:2], x8[0:1, 0, 0, 0:2])
            if pair == 1 and tqk == 0 and half == 0 and n == 0:
                pe_mm(acc[0:1, 0:2], w8b[0:1, 0, 0, 0:2])
            for kk in range(2):
                nc.tensor.matmul(
                    acc[:, :],
                    wt[:, kk, :, off:off + D],
                    x8[:, kk, :, n * 512:(n + 1) * 512],
                    start=(kk == 0), stop=(kk == 1),
                    perf_mode=PM.DoubleRow,
                )
            nc.vector.tensor_copy(
                t8[:, half, n * 512:(n + 1) * 512], acc[:, :])

    # ---------------- attention pair ---------------------------------------
    # Scores+exp stream h2-outer (all head-A j's, then head-B); the pair's
    # attn@v GROUPS (one (128,65) psum bank per (h2,sb), 4 DR jj-steps +
    # DVE recip + DVE normalize-mul) are deferred until all its exp tiles
    # exist and run as spare work inside the NEXT pair's slots.  PSUM
    # accumulation groups zero their whole 2KB bank on start, so concurrent
    # groups must own a bank: the 16 groups ping-pong through 2 banks.
    def fused_pair(pair, spare, self_h0=False):
        q8 = qk_tiles[(pair, 0)]
        k8 = qk_tiles[(pair, 1)]
        ets = {}
        slot = [0]

        def run_spare():
            s = slot[0]
            slot[0] += 1
            if s < len(spare):
                for thunk in spare[s]:
                    thunk()
            if self_h0 and 8 <= s < 16:
                k = s - 8
                group(0, k, first=(k == 0))

        res8T = rstp.tile([P, NSB, 2, D], F8, tag="rt", name=f"r8t{pair}")
        rd = rdp.tile([P, 16], F32, tag="rd", name=f"rd{pair}")

        def group(h2, sb, first=False):
            h = 2 * pair + h2
            g = h2 * NSB + sb
            pool, tg = (otp, "ot") if g % 2 == 0 else (spp, "sp")
            ot = pool.tile([P, DA], F32, tag=tg, name=f"ot{pair}_{g}")
            if first:
                # absorb the ACT tick of this pair's last relevant exp
                pe_mm(ot[0:1, 0:2], ets[(NJJ - 1, h2)][0:1, 1, 0:2])
            for jj in range(NJJ):
                nc.tensor.matmul(
                    ot[:, :],
                    ets[(jj, h2)][:, :, sb * P:(sb + 1) * P],
                    v8a[:, jj, :, h * DA:(h + 1) * DA],
                    start=(jj == 0), stop=(jj == NJJ - 1),
                    perf_mode=PM.DoubleRow,
                    skip_group_check=True,
                )
            nc.vector.reciprocal(rd[:, g:g + 1], ot[:, D:DA])
            rdb = rd[:, g:g + 1]
            rdb = bass.AP(rdb.tensor, rdb.offset, rdb.ap[:-1] + [[0, D]])
            nc.vector.tensor_tensor(
                out=res8T[:, sb, h2, :], in0=ot[:, 0:D], in1=rdb,
                op=ALU.mult)

        def transp():
            # transposes: (128s,128c)->psum fp8 (step 2); evict to res8
            tp = rot.tile([P, 2 * S], F8, tag="sc", name=f"tp{pair}")
            tpv = tp.rearrange("p (n two) -> p n two", two=2)[:, :, 0]
            for sb in range(NSB):
                nc.tensor.matmul(
                    tpv[:, sb * P:(sb + 1) * P],
                    res8T[:, sb, :, :], id8[:, :],
                    start=True, stop=True, is_transpose=True,
                    skip_group_check=True,
                )
            nc.vector.tensor_copy(res8[:, pair // 2, pair % 2, :], tpv[:, :])

        for h2 in range(2):
            for j in range(NT):
                jj, jhalf = j // 2, j % 2
                if jhalf == 0:
                    ets[(jj, h2)] = expp.tile(
                        [P, 2, S], F8, tag="et", name=f"et{pair}_{jj}_{h2}")
                sc = rot.tile([P, S], F32, tag="sc", name=f"sc{pair}_{j}_{h2}")
                if j == 0 and h2 == 0:
                    # absorb DVE tick of this pair's q8/k8 evicts
                    pe_mm(sc[0:1, 0:2], k8[0:1, 1, 0:2])
                for n in range(2):
                    nc.tensor.matmul(
                        sc[:, n * 512:(n + 1) * 512],
                        k8[32 * h2:32 * h2 + 32, :, j * P:(j + 1) * P],
                        q8[32 * h2:32 * h2 + 32, :, n * 512:(n + 1) * 512],
                        start=True, stop=True,
                        perf_mode=PM.DoubleRow,
                    )
                nc.scalar.activation(
                    ets[(jj, h2)][:, jhalf, :], sc[:, :], AF.Exp,
                    bias=bof[:, 4:5], scale=EXP_SCALE,
                )
                run_spare()

        out = []
        h2s = [1] if self_h0 else [0, 1]
        for h2 in h2s:
            for sb in range(NSB):
                out.append(lambda h2=h2, sb=sb,
                           f=(h2 == h2s[0] and sb == 0 and not self_h0):
                           group(h2, sb, first=f))
        out.append(transp)
        return out

    # ---------------- schedule ---------------------------------------------
    # prologue: pair-0 q/k pieces + first v tiles
    p1qk_piece(0, 0, 0)
    p1qk_piece(0, 0, 1)
    p1qk_piece(0, 1, 0)
    p1qk_piece(0, 1, 1)

    def mk_slots(n):
        return [[] for _ in range(n)]

    # pair 0 spare: v tiles + pair-1 pieces
    sp0 = mk_slots(16)
    for j in range(NT):
        sp0[j].append(lambda j=j: p1v_jtile(j))
    for i in range(4):
        sp0[8 + i].append(lambda i=i: p1qk_piece(1, i // 2, i % 2))

    def id8_warm():
        # dummy transpose: absorbs the id8 DMA tick into the PE clock so
        # real transposes carry only their single DVE wait
        td = otp.tile([P, 2 * P], F8, tag="ot", name="tdum")
        tdv = td.rearrange("p (n two) -> p n two", two=2)[:, :, 0]
        nc.tensor.matmul(tdv, id8[:, :], id8[:, :],
                         start=True, stop=True, is_transpose=True,
                         skip_group_check=True)
    sp0[12].append(id8_warm)
    fin0 = fused_pair(0, sp0)

    # pair 1: pair-0 finish work (16 groups + transp) + pair-2 pieces
    sp1 = mk_slots(16)
    for k in range(16):
        sp1[k].append(fin0[k])
    sp1[15].append(fin0[16])
    for i in range(4):
        sp1[2 + 3 * (i % 2) + (i // 2)].append(
            lambda i=i: p1qk_piece(2, i // 2, i % 2))
    fin1 = fused_pair(1, sp1)

    sp2 = mk_slots(16)
    for k in range(16):
        sp2[k].append(fin1[k])
    sp2[15].append(fin1[16])
    for i in range(4):
        sp2[2 + 3 * (i % 2) + (i // 2)].append(
            lambda i=i: p1qk_piece(3, i // 2, i % 2))
    fin2 = fused_pair(2, sp2)

    # pair 3: pair-2 finish in slots 0..7 (2/slot); pair-3 head-A groups
    # self-schedule into slots 8..15 once its h0 exps are done
    sp3 = mk_slots(16)
    for k in range(16):
        sp3[k // 2].append(fin2[k])
    sp3[7].append(fin2[16])
    fin3 = fused_pair(3, sp3, self_h0=True)
    # tail: pair-3 head-B groups + its transposes
    for thunk in fin3:
        thunk()
    if dbg is not None:
        nc.sync.dma_start(out=dbg["q8"], in_=qk_tiles[(0, 0)][:, :, :])
        nc.sync.dma_start(out=dbg["k8"], in_=qk_tiles[(0, 1)][:, :, :])
        nc.sync.dma_start(out=dbg["v8a"], in_=v8a[:, :, :, :])
        nc.sync.dma_start(out=dbg["res8"], in_=res8[:, :, :, :])

    # ---------------- P4 + bias + residual + DMA out ------------------------
    # absorb xt/bof DMA ticks into the DVE clock (plain copies tolerate
    # multi-waits; TensorScalarPtr does not)
    dve_sync(xt[0:1, 0, 0:8], bof[0:1, 0:4])
    for m in range(4):
        acc = rot.tile([P, S], F32, tag="sc", name=f"p4acc{m}")
        if m == 0:
            pe_mm(acc[0:1, 0:2], res8[0:1, 1, 1, 0:2])
            pe_mm(acc[0:1, 0:2], w8o[0:1, 0, 0, 0:2])
        for n in range(2):
            for kk in range(2):
                nc.tensor.matmul(
                    acc[:, n * 512:(n + 1) * 512],
                    w8o[:, kk, :, m * P:(m + 1) * P],
                    res8[:, kk, :, n * 512:(n + 1) * 512],
                    start=(kk == 0), stop=(kk == 1),
                    perf_mode=PM.DoubleRow,
                )
        nc.vector.scalar_tensor_tensor(
            ybig[:, m, :], acc[:, :], bof[:, m:m + 1],
            xt[:, m, :], op0=ALU.add, op1=ALU.add,
        )
        yr = y.rearrange("(k p) s -> p k s", p=P)
        if m % 2 == 0:
            nc.gpsimd.tensor_copy(scr8[0:1, m * 8:m * 8 + 8], ybig[0:1, m, 0:8])
            nc.gpsimd.dma_start(out=yr[:, m:m + 1, :], in_=ybig[:, m:m + 1, :])
        else:
            nc.sync.dma_start(out=yr[:, m:m + 1, :], in_=ybig[:, m:m + 1, :])


def build_nc():
    _install_drain_split()
    nc = bass.Bass(trn_type="TRN2", debug=False, num_devices=8)
    x_d = nc.dram_tensor("x", [C, S], F32, kind="ExternalInput")
    x8_d = nc.dram_tensor("x8", [P, 2, 2, S], F8, kind="ExternalInput")
    w8a_d = nc.dram_tensor("w8a", [P, 2, 2, 256], F8, kind="ExternalInput")
    w8b_d = nc.dram_tensor("w8b", [P, 2, 2, 768], F8, kind="ExternalInput")
    w8v_d = nc.dram_tensor("w8v", [P, 2, 2, 512], F8, kind="ExternalInput")
    w8o_d = nc.dram_tensor("w8o", [P, 2, 2, 512], F8, kind="ExternalInput")
    id8_d = nc.dram_tensor("id8", [P, P], F8, kind="ExternalInput")
    bof_d = nc.dram_tensor("bof", [P, 5], F32, kind="ExternalInput")
    y_d = nc.dram_tensor("y", [C, S], F32, kind="ExternalOutput")
    dbg = None
    if DEBUG_DUMP:
        dbg = {
            "q8": nc.dram_tensor("dbg_q8", [D, 2, S], F8, kind="ExternalOutput").ap(),
            "k8": nc.dram_tensor("dbg_k8", [D, 2, S], F8, kind="ExternalOutput").ap(),
            "v8a": nc.dram_tensor("dbg_v8a", [P, NJJ, 2, NH * DA], F8, kind="ExternalOutput").ap(),
            "res8": nc.dram_tensor("dbg_res8", [P, 2, 2, S], F8, kind="ExternalOutput").ap(),
        }
    with tile.TileContext(nc) as tc, ExitStack() as ctx:
        trace_kernel(ctx, tc, nc, x_d.ap(), x8_d.ap(),
                     (w8a_d.ap(), w8b_d.ap(), w8v_d.ap(), w8o_d.ap()),
                     id8_d.ap(), bof_d.ap(), y_d.ap(), dbg)
    _strip_self_waits(nc)
    if not nc.is_finalized():
        nc.finalize()
    return nc


def host_inputs(x, Wqkv, Wo, bo):
    """Host-side reshard + fp8 quantization (weights replicated)."""
    f8 = ml_dtypes.float8_e4m3fn
    x = np.ascontiguousarray(np.asarray(x, dtype=np.float32))
    Wqkv = np.asarray(Wqkv, dtype=np.float32)
    Wo = np.asarray(Wo, dtype=np.float32)
    bo = np.asarray(bo, dtype=np.float32)

    # Wqkv rows per head h: [h*192, +64) = q, [+64, +128) = k, [+128, +192) = v
    wbig = np.empty((OTOT, C), dtype=np.float32)
    for pair in range(NPAIR):
        hA, hB = 2 * pair, 2 * pair + 1
        base = pair * 256
        for tqk, roff in ((0, 0), (1, D)):          # q rows, then k rows
            for half in range(2):
                o = base + tqk * 128 + half * D
                rA = hA * 192 + roff + half * 32
                rB = hB * 192 + roff + half * 32
                wbig[o:o + 32] = Wqkv[rA:rA + 32]
                wbig[o + 32:o + 64] = Wqkv[rB:rB + 32]
    for h in range(NH):
        wbig[OV + h * D:OV + (h + 1) * D] = Wqkv[h * 192 + 128:h * 192 + 192]
    wbig[OWO:OWO + C] = Wo
    # w8[p, kk, i, o] = wbig[o, (2kk+i)*128+p], shipped as 4 contiguous
    # section tensors so each DMA sprays across all engines
    w8 = wbig.T.reshape(2, 2, P, OTOT).transpose(2, 0, 1, 3).astype(f8)
    w8a = np.ascontiguousarray(w8[:, :, :, 0:256])
    w8b = np.ascontiguousarray(w8[:, :, :, 256:OV])
    w8v = np.ascontiguousarray(w8[:, :, :, OV:OV + 512])
    w8o = np.ascontiguousarray(w8[:, :, :, OWO:OTOT])
    id8 = np.eye(P, dtype=np.float32).astype(f8)
    bof = np.concatenate(
        [bo.reshape(4, P).T, np.full((P, 1), EXP_SHIFT, np.float32)], axis=1)
    bof = np.ascontiguousarray(bof)

    ins = []
    for b in range(B):
        xs = np.ascontiguousarray(x[b].reshape(C, S))
        x8 = np.ascontiguousarray(
            xs.reshape(2, 2, P, S).transpose(2, 0, 1, 3)).astype(f8)
        ins.append(dict(x=xs, x8=x8, w8a=w8a, w8b=w8b, w8v=w8v, w8o=w8o,
                        id8=id8, bof=bof))
    return ins


_NC_CACHE = []

try:
    import jax as _jax

    _jax.clear_caches()
except Exception:
    pass


def get_nc():
    if not _NC_CACHE:
        _NC_CACHE.append(build_nc())
    return _NC_CACHE[0]


def run(in_maps, **kwargs):
    return run_bass_kernel_spmd(get_nc(), in_maps, core_ids=list(range(B)), **kwargs)


def kernel(x, Wqkv, Wo, bo):
    in_maps = host_inputs(x, Wqkv, Wo, bo)
    r = run(in_maps)
    y = np.stack([r.results[b]["y"].reshape(C, H, W) for b in range(B)])
    return y.astype(np.float32)


if __name__ == "__main__":
    nc = build_nc()
    print("built ok:", len(nc.inst_map), "instructions")


# revision 6
# speedup vs baseline: 1.4604x; 1.0494x over previous
"""Trainium2 Bass kernel v3 for nn_AttentionBlock (B=8, C=512, H=W=32, 8 heads).

Sharding: data-parallel over batch (core b owns image b; weights replicated).

All heavy matmuls run as fp8e4m3 DoubleRow (0.5 cycles/row, 2 stacked
K-planes per instruction); softmax exp runs on ACT with fp8 output and a
constant -1 logit shift (softmax-invariant) so exp() fits e4m3 range.

Per-core pipeline (x viewed as (C=512, S=1024)):
  P1qk: W-piece DR matmuls emit q,k d-SPLIT: q8/k8 (64p, 2, S) fp8 where
        partition p<32 is head A, 32<=p<64 head B, plane i = d-half.  This
        costs 2x the minimal P1 instruction count but enables...
  P2  : scores DR: lhsT=k8 (K=32, planes=d-halves) -> scoresT (128t, S) psum,
        at 0.5 cyc/row with no repacking.
  exp : ACT exp(0.125*sc - 1) -> et8 fp8 tiles (128t, 2, S); the 2-plane
        j-PAIR layout feeds attn@v DR directly.
  P3  : attn@v TALL: out(s-block 128, 65) = et8^T @ [v|1]: the 65-col output
        orientation costs out-free=65 per instruction and lands the softmax
        denominator as a per-partition COLUMN (col 64).
  norm: DVE reciprocal (free=4) + stride-0-broadcast tensor_tensor multiply
        -> res8T (s-part) fp8; PE fp8 transposes (via identity) flip it back
        to channel-partitions for...
  P4  : output projection DR + bias + residual (DVE stt) -> DMA out.
"""

import os
import sys

for _p in ("/opt/trn_rl_repo", "/root/.axon_site/_ro/trn_rl_repo"):
    if os.path.isdir(_p) and _p not in sys.path:
        sys.path.insert(0, _p)

from contextlib import ExitStack

import numpy as np
import ml_dtypes

import concourse.bass as bass
import concourse.tile as tile
from concourse import mybir
from concourse.bass_utils import run_bass_kernel_spmd

B, C, H, W = 8, 512, 32, 32
NH, D = 8, 64
S = H * W            # 1024
P = 128
NPAIR = NH // 2      # 4
NT = 8               # t-tiles (128 each)
NJJ = 4              # j-pairs
NSB = 8              # s-blocks for tall attn@v
DA = D + 1           # 65 = v cols + ones col
OQK, OV, OWO = 0, 1024, 1536   # w8 column sections
OTOT = 2048
F32 = mybir.dt.float32
F8 = mybir.dt.float8e4
BF16 = mybir.dt.bfloat16
AF = mybir.ActivationFunctionType
ALU = mybir.AluOpType
PM = mybir.MatmulPerfMode
EXP_SHIFT = -1.0
EXP_SCALE = 1.0 / np.sqrt(D)
DEBUG_DUMP = os.environ.get("K3_DEBUG", "0") == "1"


def _install_drain_split():
    """walrus's CTRL_NO (drain) codegen accepts only a single semaphore wait,
    but Tile's kernel-tail drain aggregates one wait per live proc.  Split
    them across several serial drains."""
    if getattr(tile.TileContext, "_drain_split_installed", False):
        return
    from concourse.vector_clock import ScopedClock

    orig = tile.TileContext._drain_and_barrier

    def patched(self, tick_clock, wait_clock):
        nc = self.nc
        drain_inst = nc.sync.drain()
        wait_clock.add_sem_waits(
            drain_inst.ins, ScopedClock({None: tick_clock.global_clock})
        )
        si = drain_inst.ins.sync_info
        if si is not None and si.on_wait and len(si.on_wait) > 1:
            waits = list(si.on_wait)
            drain_inst.ins.sync_info = mybir.SyncInfo(
                on_wait=[waits[0]], on_update=list(si.on_update or [])
            )
            for w in waits[1:]:
                d2 = nc.sync.drain()
                d2.ins.sync_info = mybir.SyncInfo(on_wait=[w], on_update=[])

        nc.all_engine_barrier()
        assert self.sems is not None
        popped = nc._tile_sem_poison_stack.pop()
        assert popped is self._sem_poison
        nc.clear_and_free_semaphores(list(self.sems.allocated().values()))
        nc.all_engine_barrier()

    tile.TileContext._drain_and_barrier = patched
    tile.TileContext._drain_split_installed = True
    tile.TileContext._drain_and_barrier_orig = orig


ENGINE_SEM_PREFIX = {
    "PE": "PE_",
    "Activation": "Activation_",
    "DVE": "DVE_",
    "Pool": "Pool_",
    "SP": "SP_",
}


def _strip_self_waits(nc):
    """Drop same-engine semaphore self-waits from multi-wait instructions
    (engines complete their own instructions in program order)."""
    n = 0
    for inst in nc.inst_map.values():
        si = getattr(inst, "sync_info", None)
        if si is None or not si.on_wait or len(si.on_wait) <= 1:
            continue
        eng = str(getattr(inst, "engine", "")).split(".")[-1]
        pref = ENGINE_SEM_PREFIX.get(eng)
        if pref is None:
            continue
        keep = [w for w in si.on_wait if not w.ant_name.startswith(pref)]
        if len(keep) != len(si.on_wait) and keep:
            inst.sync_info = mybir.SyncInfo(
                on_wait=keep, on_update=list(si.on_update or [])
            )
            n += 1
    return n


def trace_kernel(ctx, tc, nc, x, x8d, w8d, id8d, bofd, y, dbg=None):
    cst = ctx.enter_context(tc.tile_pool(name="cst", bufs=1))
    qkp = ctx.enter_context(tc.tile_pool(name="qkp", bufs=4))
    expp = ctx.enter_context(tc.tile_pool(name="expp", bufs=16))
    rstp = ctx.enter_context(tc.tile_pool(name="rstp", bufs=2))
    rdp = ctx.enter_context(tc.tile_pool(name="rdp", bufs=2))
    yp = ctx.enter_context(tc.tile_pool(name="yp", bufs=1))
    rot = ctx.enter_context(tc.tile_pool(name="rot", bufs=3, space="PSUM"))
    spp = ctx.enter_context(tc.tile_pool(name="spp", bufs=1, space="PSUM"))
    otp = ctx.enter_context(tc.tile_pool(name="otp", bufs=1, space="PSUM"))

    xt = cst.tile([P, 4, S], BF16)
    x8 = cst.tile([P, 2, 2, S], F8)
    w8a = cst.tile([P, 2, 2, 256], F8)
    w8b = cst.tile([P, 2, 2, 768], F8)
    w8v = cst.tile([P, 2, 2, 512], F8)
    w8o = cst.tile([P, 2, 2, 512], F8)
    id8 = cst.tile([P, P], F8)
    bof = cst.tile([P, 5], F32)
    v8a = cst.tile([P, NJJ, 2, NH * DA], F8)
    res8 = cst.tile([P, 2, 2, S], F8)
    scr = cst.tile([1, 256], F32)
    scr8 = cst.tile([1, 64], F32)
    ybig = yp.tile([P, 4, S], BF16)

    w8a_d, w8b_d, w8v_d, w8o_d = w8d
    nc.gpsimd.dma_start(out=w8a[:, :, :, :], in_=w8a_d)
    nc.sync.dma_start(out=x8[:, :, :, :], in_=x8d)
    nc.gpsimd.dma_start(out=bof[:, :], in_=bofd)
    nc.gpsimd.dma_start(out=w8v[:, :, :, :], in_=w8v_d)
    nc.sync.dma_start(out=w8b[:, :, :, :], in_=w8b_d)
    nc.gpsimd.dma_start(out=w8o[:, :, :, :], in_=w8o_d)
    nc.gpsimd.dma_start(out=id8[:, :], in_=id8d)
    nc.sync.dma_start(out=xt[:, :, :], in_=x.rearrange("(k p) s -> p k s", p=P))

    scr_i = [0]

    def dve_sync(*aps):
        # DVE wait-carrier: absorb one cross-engine wait per tiny copy.
        for ap in aps:
            n = min(ap.free_size(), 8)
            o = (scr_i[0] % 30) * 8
            scr_i[0] += 1
            nc.vector.tensor_copy(scr[0:1, o:o + n], ap)

    def pe_mm(corner, dep):
        # PE wait-carrier: a 1x2 matmul reading `dep` absorbs one cross-
        # engine wait; PE program order subsumes the tick for later matmuls.
        nc.tensor.matmul(
            corner, dep[:, 0:1], dep[:, 0:2],
            start=True, stop=True, skip_group_check=True,
        )

    # ones columns of v8a (fp8 1.0 via exp(0*x)), one ACT instr per jj;
    # reading bof also absorbs its DMA tick into the ACT clock (the exp
    # instrs' bias operand then needs no extra wait)
    # tiny ACT read of bof first: absorbs its DMA tick into the ACT clock
    dum = cst.tile([P, 1], F32)
    nc.scalar.activation(dum[:, :], bof[:, 4:5], AF.Exp, scale=0.0)
    for jj in range(NJJ):
        nc.scalar.activation(
            v8a.rearrange("p j i (h e) -> p j i h e", h=NH)[:, jj, :, :, D:DA],
            x8[:, 0, :, jj * NH:(jj + 1) * NH][:, :, :, None],
            AF.Exp, scale=0.0,
        )

    # ---------------- P1: v projection (tall: t on partitions) -------------
    # v8a[p, jj, i, h*65+e] ; v-acc psum (128t, 512 vchans)
    def p1v_jtile(j):
        acc = rot.tile([P, 512], F32, tag="sc", name=f"vacc{j}")
        if j == 0:
            pe_mm(acc[0:1, 0:2], w8v[0:1, 0, 0, 0:2])
        for kk in range(2):
            nc.tensor.matmul(
                acc[:, :],
                x8[:, kk, :, j * P:(j + 1) * P],
                w8v[:, kk, :, :],
                start=(kk == 0), stop=(kk == 1),
                perf_mode=PM.DoubleRow,
            )
        nc.vector.tensor_copy(
            v8a[:, j // 2, j % 2, :].rearrange("p (h e) -> p h e", h=NH)[:, :, 0:D],
            acc.rearrange("p (h d) -> p h d", h=NH),
        )

    # ---------------- P1: q/k pieces (d-split, M=64) ------------------------
    qk_tiles = {}

    def p1qk_piece(pair, tqk, half):
        """One M=64 piece: rows = [head 2pair (32) | head 2pair+1 (32)] of
        q-half or k-half `half`; accumulates 2 DR k-steps; evicts to fp8."""
        key = (pair, tqk)
        if key not in qk_tiles:
            qk_tiles[key] = qkp.tile(
                [D, 2, S], F8, tag="qk", name=f"qk{pair}_{tqk}")
        t8 = qk_tiles[key]
        wt = w8a if pair == 0 else w8b
        off = (pair - (0 if pair == 0 else 1)) * 256 \
            + (0 if tqk == 0 else 128) + half * D
        for n in range(2):
            acc = spp.tile([D, 512], F32, tag="sp",
                           name=f"qkacc{pair}_{tqk}_{half}_{n}")
            if pair == 0 and tqk == 0 and half == 0 and n == 0:
                pe_mm(acc[0:1, 0:2], w8a[0:1, 0, 0, 0:2])
                pe_mm(acc[0:1, 0:2], x8[0:1, 0, 0, 0:2])
            if pair == 1 and tqk == 0 and half == 0 and n == 0:
                pe_mm(acc[0:1, 0:2], w8b[0:1, 0, 0, 0:2])
            for kk in range(2):
                nc.tensor.matmul(
                    acc[:, :],
                    wt[:, kk, :, off:off + D],
                    x8[:, kk, :, n * 512:(n + 1) * 512],
                    start=(kk == 0), stop=(kk == 1),
                    perf_mode=PM.DoubleRow,
                )
            nc.vector.tensor_copy(
                t8[:, half, n * 512:(n + 1) * 512], acc[:, :])

    # ---------------- attention pair ---------------------------------------
    # Scores+exp stream h2-outer (all head-A j's, then head-B); the pair's
    # attn@v GROUPS (one (128,65) psum bank per (h2,sb), 4 DR jj-steps +
    # DVE recip + DVE normalize-mul) are deferred until all its exp tiles
    # exist and run as spare work inside the NEXT pair's slots.  PSUM
    # accumulation groups zero their whole 2KB bank on start, so concurrent
    # groups must own a bank: the 16 groups ping-pong through 2 banks.
    def fused_pair(pair, spare, self_h0=False):
        q8 = qk_tiles[(pair, 0)]
        k8 = qk_tiles[(pair, 1)]
        ets = {}
        slot = [0]

        def run_spare():
            s = slot[0]
            slot[0] += 1
            if s < len(spare):
                for thunk in spare[s]:
                    thunk()
            if self_h0 and 8 <= s < 16:
                k = s - 8
                group(0, k, first=(k == 0))

        res8T = rstp.tile([P, NSB, 2, D], F8, tag="rt", name=f"r8t{pair}")
        rd = rdp.tile([P, 16], F32, tag="rd", name=f"rd{pair}")

        def group(h2, sb, first=False):
            h = 2 * pair + h2
            g = h2 * NSB + sb
            pool, tg = (otp, "ot") if g % 2 == 0 else (spp, "sp")
            ot = pool.tile([P, DA], F32, tag=tg, name=f"ot{pair}_{g}")
            if first:
                # absorb the ACT tick of this pair's last relevant exp
                pe_mm(ot[0:1, 0:2], ets[(NJJ - 1, h2)][0:1, 1, 0:2])
            for jj in range(NJJ):
                nc.tensor.matmul(
                    ot[:, :],
                    ets[(jj, h2)][:, :, sb * P:(sb + 1) * P],
                    v8a[:, jj, :, h * DA:(h + 1) * DA],
                    start=(jj == 0), stop=(jj == NJJ - 1),
                    perf_mode=PM.DoubleRow,
                    skip_group_check=True,
                )
            nc.vector.reciprocal(rd[:, g:g + 1], ot[:, D:DA])
            rdb = rd[:, g:g + 1]
            rdb = bass.AP(rdb.tensor, rdb.offset, rdb.ap[:-1] + [[0, D]])
            nc.vector.tensor_tensor(
                out=res8T[:, sb, h2, :], in0=ot[:, 0:D], in1=rdb,
                op=ALU.mult)

        def transp():
            # transposes: (128s,128c)->psum fp8 (step 2); evict to res8
            tp = rot.tile([P, 2 * S], F8, tag="sc", name=f"tp{pair}")
            tpv = tp.rearrange("p (n two) -> p n two", two=2)[:, :, 0]
            for sb in range(NSB):
                nc.tensor.matmul(
                    tpv[:, sb * P:(sb + 1) * P],
                    res8T[:, sb, :, :], id8[:, :],
                    start=True, stop=True, is_transpose=True,
                    skip_group_check=True,
                )
            nc.vector.tensor_copy(res8[:, pair // 2, pair % 2, :], tpv[:, :])

        for h2 in range(2):
            for j in range(NT):
                jj, jhalf = j // 2, j % 2
                if jhalf == 0:
                    ets[(jj, h2)] = expp.tile(
                        [P, 2, S], F8, tag="et", name=f"et{pair}_{jj}_{h2}")
                sc = rot.tile([P, S], F32, tag="sc", name=f"sc{pair}_{j}_{h2}")
                if j == 0 and h2 == 0:
                    # absorb DVE tick of this pair's q8/k8 evicts
                    pe_mm(sc[0:1, 0:2], k8[0:1, 1, 0:2])
                for n in range(2):
                    nc.tensor.matmul(
                        sc[:, n * 512:(n + 1) * 512],
                        k8[32 * h2:32 * h2 + 32, :, j * P:(j + 1) * P],
                        q8[32 * h2:32 * h2 + 32, :, n * 512:(n + 1) * 512],
                        start=True, stop=True,
                        perf_mode=PM.DoubleRow,
                    )
                nc.scalar.activation(
                    ets[(jj, h2)][:, jhalf, :], sc[:, :], AF.Exp,
                    bias=bof[:, 4:5], scale=EXP_SCALE,
                )
                run_spare()

        out = []
        h2s = [1] if self_h0 else [0, 1]
        for h2 in h2s:
            for sb in range(NSB):
                out.append(lambda h2=h2, sb=sb,
                           f=(h2 == h2s[0] and sb == 0 and not self_h0):
                           group(h2, sb, first=f))
        out.append(transp)
        return out

    # ---------------- schedule ---------------------------------------------
    # prologue: pair-0 q/k pieces + first v tiles
    p1qk_piece(0, 0, 0)
    p1qk_piece(0, 0, 1)
    p1qk_piece(0, 1, 0)
    p1qk_piece(0, 1, 1)

    def mk_slots(n):
        return [[] for _ in range(n)]

    # pair 0 spare: v tiles + pair-1 pieces
    sp0 = mk_slots(16)
    for j in range(NT):
        sp0[j].append(lambda j=j: p1v_jtile(j))
    for i in range(4):
        sp0[8 + i].append(lambda i=i: p1qk_piece(1, i // 2, i % 2))

    def id8_warm():
        # dummy transpose: absorbs the id8 DMA tick into the PE clock so
        # real transposes carry only their single DVE wait
        td = otp.tile([P, 2 * P], F8, tag="ot", name="tdum")
        tdv = td.rearrange("p (n two) -> p n two", two=2)[:, :, 0]
        nc.tensor.matmul(tdv, id8[:, :], id8[:, :],
                         start=True, stop=True, is_transpose=True,
                         skip_group_check=True)
    sp0[12].append(id8_warm)
    fin0 = fused_pair(0, sp0)

    # pair 1: pair-0 finish work (16 groups + transp) + pair-2 pieces
    sp1 = mk_slots(16)
    for k in range(16):
        sp1[k].append(fin0[k])
    sp1[15].append(fin0[16])
    for i in range(4):
        sp1[2 + 3 * (i % 2) + (i // 2)].append(
            lambda i=i: p1qk_piece(2, i // 2, i % 2))
    fin1 = fused_pair(1, sp1)

    sp2 = mk_slots(16)
    for k in range(16):
        sp2[k].append(fin1[k])
    sp2[15].append(fin1[16])
    for i in range(4):
        sp2[2 + 3 * (i % 2) + (i // 2)].append(
            lambda i=i: p1qk_piece(3, i // 2, i % 2))
    fin2 = fused_pair(2, sp2)

    # pair 3: pair-2 finish in slots 0..7 (2/slot); pair-3 head-A groups
    # self-schedule into slots 8..15 once its h0 exps are done
    sp3 = mk_slots(16)
    for k in range(16):
        sp3[k // 2].append(fin2[k])
    sp3[7].append(fin2[16])
    fin3 = fused_pair(3, sp3, self_h0=True)
    # tail: pair-3 head-B groups + its transposes
    for thunk in fin3:
        thunk()
    if dbg is not None:
        nc.sync.dma_start(out=dbg["q8"], in_=qk_tiles[(0, 0)][:, :, :])
        nc.sync.dma_start(out=dbg["k8"], in_=qk_tiles[(0, 1)][:, :, :])
        nc.sync.dma_start(out=dbg["v8a"], in_=v8a[:, :, :, :])
        nc.sync.dma_start(out=dbg["res8"], in_=res8[:, :, :, :])

    # ---------------- P4 + bias + residual + DMA out ------------------------
    # absorb xt/bof DMA ticks into the DVE clock (plain copies tolerate
    # multi-waits; TensorScalarPtr does not)
    dve_sync(xt[0:1, 0, 0:8], bof[0:1, 0:4])
    for m in range(4):
        acc = rot.tile([P, S], F32, tag="sc", name=f"p4acc{m}")
        if m == 0:
            pe_mm(acc[0:1, 0:2], res8[0:1, 1, 1, 0:2])
            pe_mm(acc[0:1, 0:2], w8o[0:1, 0, 0, 0:2])
        for n in range(2):
            for kk in range(2):
                nc.tensor.matmul(
                    acc[:, n * 512:(n + 1) * 512],
                    w8o[:, kk, :, m * P:(m + 1) * P],
                    res8[:, kk, :, n * 512:(n + 1) * 512],
                    start=(kk == 0), stop=(kk == 1),
                    perf_mode=PM.DoubleRow,
                )
        nc.vector.scalar_tensor_tensor(
            ybig[:, m, :], acc[:, :], bof[:, m:m + 1],
            xt[:, m, :], op0=ALU.add, op1=ALU.add,
        )
        yr = y.rearrange("(k p) s -> p k s", p=P)
        if m % 2 == 0:
            nc.gpsimd.tensor_copy(scr8[0:1, m * 8:m * 8 + 8], ybig[0:1, m, 0:8])
            nc.gpsimd.dma_start(out=yr[:, m:m + 1, :], in_=ybig[:, m:m + 1, :])
        else:
            nc.sync.dma_start(out=yr[:, m:m + 1, :], in_=ybig[:, m:m + 1, :])


def build_nc():
    _install_drain_split()
    nc = bass.Bass(trn_type="TRN2", debug=False, num_devices=8)
    x_d = nc.dram_tensor("x", [C, S], BF16, kind="ExternalInput")
    x8_d = nc.dram_tensor("x8", [P, 2, 2, S], F8, kind="ExternalInput")
    w8a_d = nc.dram_tensor("w8a", [P, 2, 2, 256], F8, kind="ExternalInput")
    w8b_d = nc.dram_tensor("w8b", [P, 2, 2, 768], F8, kind="ExternalInput")
    w8v_d = nc.dram_tensor("w8v", [P, 2, 2, 512], F8, kind="ExternalInput")
    w8o_d = nc.dram_tensor("w8o", [P, 2, 2, 512], F8, kind="ExternalInput")
    id8_d = nc.dram_tensor("id8", [P, P], F8, kind="ExternalInput")
    bof_d = nc.dram_tensor("bof", [P, 5], F32, kind="ExternalInput")
    y_d = nc.dram_tensor("y", [C, S], BF16, kind="ExternalOutput")
    dbg = None
    if DEBUG_DUMP:
        dbg = {
            "q8": nc.dram_tensor("dbg_q8", [D, 2, S], F8, kind="ExternalOutput").ap(),
            "k8": nc.dram_tensor("dbg_k8", [D, 2, S], F8, kind="ExternalOutput").ap(),
            "v8a": nc.dram_tensor("dbg_v8a", [P, NJJ, 2, NH * DA], F8, kind="ExternalOutput").ap(),
            "res8": nc.dram_tensor("dbg_res8", [P, 2, 2, S], F8, kind="ExternalOutput").ap(),
        }
    with tile.TileContext(nc) as tc, ExitStack() as ctx:
        trace_kernel(ctx, tc, nc, x_d.ap(), x8_d.ap(),
                     (w8a_d.ap(), w8b_d.ap(), w8v_d.ap(), w8o_d.ap()),
                     id8_d.ap(), bof_d.ap(), y_d.ap(), dbg)
    _strip_self_waits(nc)
    if not nc.is_finalized():
        nc.finalize()
    return nc


def host_inputs(x, Wqkv, Wo, bo):
    """Host-side reshard + fp8 quantization (weights replicated)."""
    f8 = ml_dtypes.float8_e4m3fn
    x = np.ascontiguousarray(np.asarray(x, dtype=np.float32))
    Wqkv = np.asarray(Wqkv, dtype=np.float32)
    Wo = np.asarray(Wo, dtype=np.float32)
    bo = np.asarray(bo, dtype=np.float32)

    # Wqkv rows per head h: [h*192, +64) = q, [+64, +128) = k, [+128, +192) = v
    wbig = np.empty((OTOT, C), dtype=np.float32)
    for pair in range(NPAIR):
        hA, hB = 2 * pair, 2 * pair + 1
        base = pair * 256
        for tqk, roff in ((0, 0), (1, D)):          # q rows, then k rows
            for half in range(2):
                o = base + tqk * 128 + half * D
                rA = hA * 192 + roff + half * 32
                rB = hB * 192 + roff + half * 32
                wbig[o:o + 32] = Wqkv[rA:rA + 32]
                wbig[o + 32:o + 64] = Wqkv[rB:rB + 32]
    for h in range(NH):
        wbig[OV + h * D:OV + (h + 1) * D] = Wqkv[h * 192 + 128:h * 192 + 192]
    wbig[OWO:OWO + C] = Wo
    # w8[p, kk, i, o] = wbig[o, (2kk+i)*128+p], shipped as 4 contiguous
    # section tensors so each DMA sprays across all engines
    w8 = wbig.T.reshape(2, 2, P, OTOT).transpose(2, 0, 1, 3).astype(f8)
    w8a = np.ascontiguousarray(w8[:, :, :, 0:256])
    w8b = np.ascontiguousarray(w8[:, :, :, 256:OV])
    w8v = np.ascontiguousarray(w8[:, :, :, OV:OV + 512])
    w8o = np.ascontiguousarray(w8[:, :, :, OWO:OTOT])
    id8 = np.eye(P, dtype=np.float32).astype(f8)
    bof = np.concatenate(
        [bo.reshape(4, P).T, np.full((P, 1), EXP_SHIFT, np.float32)], axis=1)
    bof = np.ascontiguousarray(bof)

    ins = []
    for b in range(B):
        xs = np.ascontiguousarray(x[b].reshape(C, S))
        x8 = np.ascontiguousarray(
            xs.reshape(2, 2, P, S).transpose(2, 0, 1, 3)).astype(f8)
        xbf = xs.astype(ml_dtypes.bfloat16)
        ins.append(dict(x=xbf, x8=x8, w8a=w8a, w8b=w8b, w8v=w8v, w8o=w8o,
                        id8=id8, bof=bof))
    return ins


_NC_CACHE = []

try:
    import jax as _jax

    _jax.clear_caches()
except Exception:
    pass


def get_nc():
    if not _NC_CACHE:
        _NC_CACHE.append(build_nc())
    return _NC_CACHE[0]


def run(in_maps, **kwargs):
    return run_bass_kernel_spmd(get_nc(), in_maps, core_ids=list(range(B)), **kwargs)


def kernel(x, Wqkv, Wo, bo):
    in_maps = host_inputs(x, Wqkv, Wo, bo)
    r = run(in_maps)
    y = np.stack([r.results[b]["y"].reshape(C, H, W) for b in range(B)])
    return y.astype(np.float32)


if __name__ == "__main__":
    nc = build_nc()
    print("built ok:", len(nc.inst_map), "instructions")


# revision 7
# speedup vs baseline: 1.4624x; 1.0014x over previous
"""Trainium2 Bass kernel v3 for nn_AttentionBlock (B=8, C=512, H=W=32, 8 heads).

Sharding: data-parallel over batch (core b owns image b; weights replicated).

All heavy matmuls run as fp8e4m3 DoubleRow (0.5 cycles/row, 2 stacked
K-planes per instruction); softmax exp runs on ACT with fp8 output and a
constant -1 logit shift (softmax-invariant) so exp() fits e4m3 range.

Per-core pipeline (x viewed as (C=512, S=1024)):
  P1qk: W-piece DR matmuls emit q,k d-SPLIT: q8/k8 (64p, 2, S) fp8 where
        partition p<32 is head A, 32<=p<64 head B, plane i = d-half.  This
        costs 2x the minimal P1 instruction count but enables...
  P2  : scores DR: lhsT=k8 (K=32, planes=d-halves) -> scoresT (128t, S) psum,
        at 0.5 cyc/row with no repacking.
  exp : ACT exp(0.125*sc - 1) -> et8 fp8 tiles (128t, 2, S); the 2-plane
        j-PAIR layout feeds attn@v DR directly.
  P3  : attn@v TALL: out(s-block 128, 65) = et8^T @ [v|1]: the 65-col output
        orientation costs out-free=65 per instruction and lands the softmax
        denominator as a per-partition COLUMN (col 64).
  norm: DVE reciprocal (free=4) + stride-0-broadcast tensor_tensor multiply
        -> res8T (s-part) fp8; PE fp8 transposes (via identity) flip it back
        to channel-partitions for...
  P4  : output projection DR + bias + residual (DVE stt) -> DMA out.
"""

import os
import sys

for _p in ("/opt/trn_rl_repo", "/root/.axon_site/_ro/trn_rl_repo"):
    if os.path.isdir(_p) and _p not in sys.path:
        sys.path.insert(0, _p)

from contextlib import ExitStack

import numpy as np
import ml_dtypes

import concourse.bass as bass
import concourse.tile as tile
from concourse import mybir
from concourse.bass_utils import run_bass_kernel_spmd

B, C, H, W = 8, 512, 32, 32
NH, D = 8, 64
S = H * W            # 1024
P = 128
NPAIR = NH // 2      # 4
NT = 8               # t-tiles (128 each)
NJJ = 4              # j-pairs
NSB = 8              # s-blocks for tall attn@v
DA = D + 1           # 65 = v cols + ones col
OQK, OV, OWO = 0, 1024, 1536   # w8 column sections
OTOT = 2048
F32 = mybir.dt.float32
F8 = mybir.dt.float8e4
BF16 = mybir.dt.bfloat16
AF = mybir.ActivationFunctionType
ALU = mybir.AluOpType
PM = mybir.MatmulPerfMode
EXP_SHIFT = -1.0
EXP_SCALE = 1.0 / np.sqrt(D)
DEBUG_DUMP = os.environ.get("K3_DEBUG", "0") == "1"


def _install_drain_split():
    """walrus's CTRL_NO (drain) codegen accepts only a single semaphore wait,
    but Tile's kernel-tail drain aggregates one wait per live proc.  Split
    them across several serial drains."""
    if getattr(tile.TileContext, "_drain_split_installed", False):
        return
    from concourse.vector_clock import ScopedClock

    orig = tile.TileContext._drain_and_barrier

    def patched(self, tick_clock, wait_clock):
        nc = self.nc
        drain_inst = nc.sync.drain()
        wait_clock.add_sem_waits(
            drain_inst.ins, ScopedClock({None: tick_clock.global_clock})
        )
        si = drain_inst.ins.sync_info
        if si is not None and si.on_wait and len(si.on_wait) > 1:
            waits = list(si.on_wait)
            drain_inst.ins.sync_info = mybir.SyncInfo(
                on_wait=[waits[0]], on_update=list(si.on_update or [])
            )
            for w in waits[1:]:
                d2 = nc.sync.drain()
                d2.ins.sync_info = mybir.SyncInfo(on_wait=[w], on_update=[])

        nc.all_engine_barrier()
        assert self.sems is not None
        popped = nc._tile_sem_poison_stack.pop()
        assert popped is self._sem_poison
        nc.clear_and_free_semaphores(list(self.sems.allocated().values()))
        nc.all_engine_barrier()

    tile.TileContext._drain_and_barrier = patched
    tile.TileContext._drain_split_installed = True
    tile.TileContext._drain_and_barrier_orig = orig


ENGINE_SEM_PREFIX = {
    "PE": "PE_",
    "Activation": "Activation_",
    "DVE": "DVE_",
    "Pool": "Pool_",
    "SP": "SP_",
}


def _strip_self_waits(nc):
    """Drop same-engine semaphore self-waits from multi-wait instructions
    (engines complete their own instructions in program order)."""
    n = 0
    for inst in nc.inst_map.values():
        si = getattr(inst, "sync_info", None)
        if si is None or not si.on_wait or len(si.on_wait) <= 1:
            continue
        eng = str(getattr(inst, "engine", "")).split(".")[-1]
        pref = ENGINE_SEM_PREFIX.get(eng)
        if pref is None:
            continue
        keep = [w for w in si.on_wait if not w.ant_name.startswith(pref)]
        if len(keep) != len(si.on_wait) and keep:
            inst.sync_info = mybir.SyncInfo(
                on_wait=keep, on_update=list(si.on_update or [])
            )
            n += 1
    return n


def trace_kernel(ctx, tc, nc, x, x8d, w8d, id8d, bofd, y, dbg=None):
    cst = ctx.enter_context(tc.tile_pool(name="cst", bufs=1))
    qkp = ctx.enter_context(tc.tile_pool(name="qkp", bufs=4))
    expp = ctx.enter_context(tc.tile_pool(name="expp", bufs=16))
    rstp = ctx.enter_context(tc.tile_pool(name="rstp", bufs=2))
    rdp = ctx.enter_context(tc.tile_pool(name="rdp", bufs=2))
    yp = ctx.enter_context(tc.tile_pool(name="yp", bufs=1))
    rot = ctx.enter_context(tc.tile_pool(name="rot", bufs=3, space="PSUM"))
    spp = ctx.enter_context(tc.tile_pool(name="spp", bufs=1, space="PSUM"))
    otp = ctx.enter_context(tc.tile_pool(name="otp", bufs=1, space="PSUM"))

    xt = cst.tile([P, 4, S], BF16)
    x8 = cst.tile([P, 2, 2, S], F8)
    w8a = cst.tile([P, 2, 2, 256], F8)
    w8b = cst.tile([P, 2, 2, 768], F8)
    w8v = cst.tile([P, 2, 2, 512], F8)
    w8o = cst.tile([P, 2, 2, 512], F8)
    id8 = cst.tile([P, P], F8)
    bof = cst.tile([P, 5], F32)
    v8a = cst.tile([P, NJJ, 2, NH * DA], F8)
    res8 = cst.tile([P, 2, 2, S], F8)
    scr = cst.tile([1, 256], F32)
    scr8 = cst.tile([1, 64], F32)
    ybig = yp.tile([P, 4, S], BF16)

    w8a_d, w8b_d, w8v_d, w8o_d = w8d
    nc.gpsimd.dma_start(out=w8a[:, :, :, :], in_=w8a_d)
    nc.sync.dma_start(out=x8[:, :, :, :], in_=x8d)
    nc.gpsimd.dma_start(out=bof[:, :], in_=bofd)
    nc.gpsimd.dma_start(out=w8v[:, :, :, :], in_=w8v_d)
    nc.sync.dma_start(out=w8b[:, :, :, :], in_=w8b_d)
    nc.gpsimd.dma_start(out=w8o[:, :, :, :], in_=w8o_d)
    nc.gpsimd.dma_start(out=id8[:, :], in_=id8d)
    nc.sync.dma_start(out=xt[:, :, :], in_=x.rearrange("(k p) s -> p k s", p=P))

    scr_i = [0]

    def dve_sync(*aps):
        # DVE wait-carrier: absorb one cross-engine wait per tiny copy.
        for ap in aps:
            n = min(ap.free_size(), 8)
            o = (scr_i[0] % 30) * 8
            scr_i[0] += 1
            nc.vector.tensor_copy(scr[0:1, o:o + n], ap)

    def pe_mm(corner, dep):
        # PE wait-carrier: a 1x2 matmul reading `dep` absorbs one cross-
        # engine wait; PE program order subsumes the tick for later matmuls.
        nc.tensor.matmul(
            corner, dep[:, 0:1], dep[:, 0:2],
            start=True, stop=True, skip_group_check=True,
        )

    # ones columns of v8a (fp8 1.0 via exp(0*x)), one ACT instr per jj;
    # reading bof also absorbs its DMA tick into the ACT clock (the exp
    # instrs' bias operand then needs no extra wait)
    # tiny ACT read of bof first: absorbs its DMA tick into the ACT clock
    dum = cst.tile([P, 1], F32)
    nc.scalar.activation(dum[:, :], bof[:, 4:5], AF.Exp, scale=0.0)
    for jj in range(NJJ):
        nc.scalar.activation(
            v8a.rearrange("p j i (h e) -> p j i h e", h=NH)[:, jj, :, :, D:DA],
            x8[:, 0, :, jj * NH:(jj + 1) * NH][:, :, :, None],
            AF.Exp, scale=0.0,
        )

    # ---------------- P1: v projection (tall: t on partitions) -------------
    # v8a[p, jj, i, h*65+e] ; v-acc psum (128t, 512 vchans)
    def p1v_jtile(j):
        acc = rot.tile([P, 512], F32, tag="sc", name=f"vacc{j}")
        if j == 0:
            pe_mm(acc[0:1, 0:2], w8v[0:1, 0, 0, 0:2])
        for kk in range(2):
            nc.tensor.matmul(
                acc[:, :],
                x8[:, kk, :, j * P:(j + 1) * P],
                w8v[:, kk, :, :],
                start=(kk == 0), stop=(kk == 1),
                perf_mode=PM.DoubleRow,
            )
        nc.vector.tensor_copy(
            v8a[:, j // 2, j % 2, :].rearrange("p (h e) -> p h e", h=NH)[:, :, 0:D],
            acc.rearrange("p (h d) -> p h d", h=NH),
        )

    # ---------------- P1: q/k pieces (d-split, M=64) ------------------------
    qk_tiles = {}

    def p1qk_piece(pair, tqk, half):
        """One M=64 piece: rows = [head 2pair (32) | head 2pair+1 (32)] of
        q-half or k-half `half`; accumulates 2 DR k-steps; evicts to fp8."""
        key = (pair, tqk)
        if key not in qk_tiles:
            qk_tiles[key] = qkp.tile(
                [D, 2, S], F8, tag="qk", name=f"qk{pair}_{tqk}")
        t8 = qk_tiles[key]
        wt = w8a if pair == 0 else w8b
        off = (pair - (0 if pair == 0 else 1)) * 256 \
            + (0 if tqk == 0 else 128) + half * D
        for n in range(2):
            acc = spp.tile([D, 512], F32, tag="sp",
                           name=f"qkacc{pair}_{tqk}_{half}_{n}")
            if pair == 0 and tqk == 0 and half == 0 and n == 0:
                pe_mm(acc[0:1, 0:2], w8a[0:1, 0, 0, 0:2])
                pe_mm(acc[0:1, 0:2], x8[0:1, 0, 0, 0:2])
            if pair == 1 and tqk == 0 and half == 0 and n == 0:
                pe_mm(acc[0:1, 0:2], w8b[0:1, 0, 0, 0:2])
            for kk in range(2):
                nc.tensor.matmul(
                    acc[:, :],
                    wt[:, kk, :, off:off + D],
                    x8[:, kk, :, n * 512:(n + 1) * 512],
                    start=(kk == 0), stop=(kk == 1),
                    perf_mode=PM.DoubleRow,
                )
            nc.vector.tensor_copy(
                t8[:, half, n * 512:(n + 1) * 512], acc[:, :])

    # ---------------- attention pair ---------------------------------------
    # Scores+exp stream h2-outer (all head-A j's, then head-B); the pair's
    # attn@v GROUPS (one (128,65) psum bank per (h2,sb), 4 DR jj-steps +
    # DVE recip + DVE normalize-mul) are deferred until all its exp tiles
    # exist and run as spare work inside the NEXT pair's slots.  PSUM
    # accumulation groups zero their whole 2KB bank on start, so concurrent
    # groups must own a bank: the 16 groups ping-pong through 2 banks.
    def fused_pair(pair, spare, self_h0=False):
        q8 = qk_tiles[(pair, 0)]
        k8 = qk_tiles[(pair, 1)]
        ets = {}
        slot = [0]

        def run_spare():
            s = slot[0]
            slot[0] += 1
            if s < len(spare):
                for thunk in spare[s]:
                    thunk()
            if self_h0 and 8 <= s < 16:
                k = s - 8
                group(0, k, first=(k == 0))

        res8T = rstp.tile([P, NSB, 2, D], F8, tag="rt", name=f"r8t{pair}")
        rd = rdp.tile([P, 16], F32, tag="rd", name=f"rd{pair}")

        def group(h2, sb, first=False, tail=False):
            h = 2 * pair + h2
            g = h2 * NSB + sb
            if tail and g % 2 == 1:
                # kernel tail: scores are done, the sc slots are free --
                # widen the group rotation so the chain pipelines deeper
                pool, tg = rot, "sc"
            else:
                pool, tg = (otp, "ot") if g % 2 == 0 else (spp, "sp")
            ot = pool.tile([P, DA], F32, tag=tg, name=f"ot{pair}_{g}")
            if first:
                # absorb the ACT tick of this pair's last relevant exp
                pe_mm(ot[0:1, 0:2], ets[(NJJ - 1, h2)][0:1, 1, 0:2])
            for jj in range(NJJ):
                nc.tensor.matmul(
                    ot[:, :],
                    ets[(jj, h2)][:, :, sb * P:(sb + 1) * P],
                    v8a[:, jj, :, h * DA:(h + 1) * DA],
                    start=(jj == 0), stop=(jj == NJJ - 1),
                    perf_mode=PM.DoubleRow,
                    skip_group_check=True,
                )
            nc.vector.reciprocal(rd[:, g:g + 1], ot[:, D:DA])
            rdb = rd[:, g:g + 1]
            rdb = bass.AP(rdb.tensor, rdb.offset, rdb.ap[:-1] + [[0, D]])
            nc.vector.tensor_tensor(
                out=res8T[:, sb, h2, :], in0=ot[:, 0:D], in1=rdb,
                op=ALU.mult)

        def transp():
            # transposes: (128s,128c)->psum fp8 (step 2); evict to res8
            tp = rot.tile([P, 2 * S], F8, tag="sc", name=f"tp{pair}")
            tpv = tp.rearrange("p (n two) -> p n two", two=2)[:, :, 0]
            for sb in range(NSB):
                nc.tensor.matmul(
                    tpv[:, sb * P:(sb + 1) * P],
                    res8T[:, sb, :, :], id8[:, :],
                    start=True, stop=True, is_transpose=True,
                    skip_group_check=True,
                )
            nc.vector.tensor_copy(res8[:, pair // 2, pair % 2, :], tpv[:, :])

        for h2 in range(2):
            for j in range(NT):
                jj, jhalf = j // 2, j % 2
                if jhalf == 0:
                    ets[(jj, h2)] = expp.tile(
                        [P, 2, S], F8, tag="et", name=f"et{pair}_{jj}_{h2}")
                sc = rot.tile([P, S], F32, tag="sc", name=f"sc{pair}_{j}_{h2}")
                if j == 0 and h2 == 0:
                    # absorb DVE tick of this pair's q8/k8 evicts
                    pe_mm(sc[0:1, 0:2], k8[0:1, 1, 0:2])
                for n in range(2):
                    nc.tensor.matmul(
                        sc[:, n * 512:(n + 1) * 512],
                        k8[32 * h2:32 * h2 + 32, :, j * P:(j + 1) * P],
                        q8[32 * h2:32 * h2 + 32, :, n * 512:(n + 1) * 512],
                        start=True, stop=True,
                        perf_mode=PM.DoubleRow,
                    )
                nc.scalar.activation(
                    ets[(jj, h2)][:, jhalf, :], sc[:, :], AF.Exp,
                    bias=bof[:, 4:5], scale=EXP_SCALE,
                )
                run_spare()

        out = []
        h2s = [1] if self_h0 else [0, 1]
        for h2 in h2s:
            for sb in range(NSB):
                out.append(lambda h2=h2, sb=sb,
                           f=(h2 == h2s[0] and sb == 0 and not self_h0):
                           group(h2, sb, first=f, tail=self_h0))
        out.append(transp)
        return out

    # ---------------- schedule ---------------------------------------------
    # prologue: pair-0 q/k pieces + first v tiles
    p1qk_piece(0, 0, 0)
    p1qk_piece(0, 0, 1)
    p1qk_piece(0, 1, 0)
    p1qk_piece(0, 1, 1)

    def mk_slots(n):
        return [[] for _ in range(n)]

    # pair 0 spare: v tiles + pair-1 pieces
    sp0 = mk_slots(16)
    for j in range(NT):
        sp0[j].append(lambda j=j: p1v_jtile(j))
    for i in range(4):
        sp0[8 + i].append(lambda i=i: p1qk_piece(1, i // 2, i % 2))

    def id8_warm():
        # dummy transpose: absorbs the id8 DMA tick into the PE clock so
        # real transposes carry only their single DVE wait
        td = otp.tile([P, 2 * P], F8, tag="ot", name="tdum")
        tdv = td.rearrange("p (n two) -> p n two", two=2)[:, :, 0]
        nc.tensor.matmul(tdv, id8[:, :], id8[:, :],
                         start=True, stop=True, is_transpose=True,
                         skip_group_check=True)
    sp0[12].append(id8_warm)
    fin0 = fused_pair(0, sp0)

    # pair 1: pair-0 finish work (16 groups + transp) + pair-2 pieces
    sp1 = mk_slots(16)
    for k in range(16):
        sp1[k].append(fin0[k])
    sp1[15].append(fin0[16])
    for i in range(4):
        sp1[2 + 3 * (i % 2) + (i // 2)].append(
            lambda i=i: p1qk_piece(2, i // 2, i % 2))
    fin1 = fused_pair(1, sp1)

    sp2 = mk_slots(16)
    for k in range(16):
        sp2[k].append(fin1[k])
    sp2[15].append(fin1[16])
    for i in range(4):
        sp2[2 + 3 * (i % 2) + (i // 2)].append(
            lambda i=i: p1qk_piece(3, i // 2, i % 2))
    fin2 = fused_pair(2, sp2)

    # pair 3: pair-2 finish in slots 0..7 (2/slot); pair-3 head-A groups
    # self-schedule into slots 8..15 once its h0 exps are done
    sp3 = mk_slots(16)
    for k in range(16):
        sp3[k // 2].append(fin2[k])
    sp3[7].append(fin2[16])
    fin3 = fused_pair(3, sp3, self_h0=True)
    # tail: pair-3 head-B groups + its transposes
    for thunk in fin3:
        thunk()
    if dbg is not None:
        nc.sync.dma_start(out=dbg["q8"], in_=qk_tiles[(0, 0)][:, :, :])
        nc.sync.dma_start(out=dbg["k8"], in_=qk_tiles[(0, 1)][:, :, :])
        nc.sync.dma_start(out=dbg["v8a"], in_=v8a[:, :, :, :])
        nc.sync.dma_start(out=dbg["res8"], in_=res8[:, :, :, :])

    # ---------------- P4 + bias + residual + DMA out ------------------------
    # absorb xt/bof DMA ticks into the DVE clock (plain copies tolerate
    # multi-waits; TensorScalarPtr does not)
    dve_sync(xt[0:1, 0, 0:8], bof[0:1, 0:4])
    for m in range(4):
        acc = rot.tile([P, S], F32, tag="sc", name=f"p4acc{m}")
        if m == 0:
            pe_mm(acc[0:1, 0:2], res8[0:1, 1, 1, 0:2])
            pe_mm(acc[0:1, 0:2], w8o[0:1, 0, 0, 0:2])
        for n in range(2):
            for kk in range(2):
                nc.tensor.matmul(
                    acc[:, n * 512:(n + 1) * 512],
                    w8o[:, kk, :, m * P:(m + 1) * P],
                    res8[:, kk, :, n * 512:(n + 1) * 512],
                    start=(kk == 0), stop=(kk == 1),
                    perf_mode=PM.DoubleRow,
                )
        nc.vector.scalar_tensor_tensor(
            ybig[:, m, :], acc[:, :], bof[:, m:m + 1],
            xt[:, m, :], op0=ALU.add, op1=ALU.add,
        )
        yr = y.rearrange("(k p) s -> p k s", p=P)
        if m % 2 == 0:
            nc.gpsimd.tensor_copy(scr8[0:1, m * 8:m * 8 + 8], ybig[0:1, m, 0:8])
            nc.gpsimd.dma_start(out=yr[:, m:m + 1, :], in_=ybig[:, m:m + 1, :])
        else:
            nc.sync.dma_start(out=yr[:, m:m + 1, :], in_=ybig[:, m:m + 1, :])


def build_nc():
    _install_drain_split()
    nc = bass.Bass(trn_type="TRN2", debug=False, num_devices=8)
    x_d = nc.dram_tensor("x", [C, S], BF16, kind="ExternalInput")
    x8_d = nc.dram_tensor("x8", [P, 2, 2, S], F8, kind="ExternalInput")
    w8a_d = nc.dram_tensor("w8a", [P, 2, 2, 256], F8, kind="ExternalInput")
    w8b_d = nc.dram_tensor("w8b", [P, 2, 2, 768], F8, kind="ExternalInput")
    w8v_d = nc.dram_tensor("w8v", [P, 2, 2, 512], F8, kind="ExternalInput")
    w8o_d = nc.dram_tensor("w8o", [P, 2, 2, 512], F8, kind="ExternalInput")
    id8_d = nc.dram_tensor("id8", [P, P], F8, kind="ExternalInput")
    bof_d = nc.dram_tensor("bof", [P, 5], F32, kind="ExternalInput")
    y_d = nc.dram_tensor("y", [C, S], BF16, kind="ExternalOutput")
    dbg = None
    if DEBUG_DUMP:
        dbg = {
            "q8": nc.dram_tensor("dbg_q8", [D, 2, S], F8, kind="ExternalOutput").ap(),
            "k8": nc.dram_tensor("dbg_k8", [D, 2, S], F8, kind="ExternalOutput").ap(),
            "v8a": nc.dram_tensor("dbg_v8a", [P, NJJ, 2, NH * DA], F8, kind="ExternalOutput").ap(),
            "res8": nc.dram_tensor("dbg_res8", [P, 2, 2, S], F8, kind="ExternalOutput").ap(),
        }
    with tile.TileContext(nc) as tc, ExitStack() as ctx:
        trace_kernel(ctx, tc, nc, x_d.ap(), x8_d.ap(),
                     (w8a_d.ap(), w8b_d.ap(), w8v_d.ap(), w8o_d.ap()),
                     id8_d.ap(), bof_d.ap(), y_d.ap(), dbg)
    _strip_self_waits(nc)
    if not nc.is_finalized():
        nc.finalize()
    return nc


def host_inputs(x, Wqkv, Wo, bo):
    """Host-side reshard + fp8 quantization (weights replicated)."""
    f8 = ml_dtypes.float8_e4m3fn
    x = np.ascontiguousarray(np.asarray(x, dtype=np.float32))
    Wqkv = np.asarray(Wqkv, dtype=np.float32)
    Wo = np.asarray(Wo, dtype=np.float32)
    bo = np.asarray(bo, dtype=np.float32)

    # Wqkv rows per head h: [h*192, +64) = q, [+64, +128) = k, [+128, +192) = v
    wbig = np.empty((OTOT, C), dtype=np.float32)
    for pair in range(NPAIR):
        hA, hB = 2 * pair, 2 * pair + 1
        base = pair * 256
        for tqk, roff in ((0, 0), (1, D)):          # q rows, then k rows
            for half in range(2):
                o = base + tqk * 128 + half * D
                rA = hA * 192 + roff + half * 32
                rB = hB * 192 + roff + half * 32
                wbig[o:o + 32] = Wqkv[rA:rA + 32]
                wbig[o + 32:o + 64] = Wqkv[rB:rB + 32]
    for h in range(NH):
        wbig[OV + h * D:OV + (h + 1) * D] = Wqkv[h * 192 + 128:h * 192 + 192]
    wbig[OWO:OWO + C] = Wo
    # w8[p, kk, i, o] = wbig[o, (2kk+i)*128+p], shipped as 4 contiguous
    # section tensors so each DMA sprays across all engines
    w8 = wbig.T.reshape(2, 2, P, OTOT).transpose(2, 0, 1, 3).astype(f8)
    w8a = np.ascontiguousarray(w8[:, :, :, 0:256])
    w8b = np.ascontiguousarray(w8[:, :, :, 256:OV])
    w8v = np.ascontiguousarray(w8[:, :, :, OV:OV + 512])
    w8o = np.ascontiguousarray(w8[:, :, :, OWO:OTOT])
    id8 = np.eye(P, dtype=np.float32).astype(f8)
    bof = np.concatenate(
        [bo.reshape(4, P).T, np.full((P, 1), EXP_SHIFT, np.float32)], axis=1)
    bof = np.ascontiguousarray(bof)

    ins = []
    for b in range(B):
        xs = np.ascontiguousarray(x[b].reshape(C, S))
        x8 = np.ascontiguousarray(
            xs.reshape(2, 2, P, S).transpose(2, 0, 1, 3)).astype(f8)
        xbf = xs.astype(ml_dtypes.bfloat16)
        ins.append(dict(x=xbf, x8=x8, w8a=w8a, w8b=w8b, w8v=w8v, w8o=w8o,
                        id8=id8, bof=bof))
    return ins


_NC_CACHE = []

try:
    import jax as _jax

    _jax.clear_caches()
except Exception:
    pass


def get_nc():
    if not _NC_CACHE:
        _NC_CACHE.append(build_nc())
    return _NC_CACHE[0]


def run(in_maps, **kwargs):
    return run_bass_kernel_spmd(get_nc(), in_maps, core_ids=list(range(B)), **kwargs)


def kernel(x, Wqkv, Wo, bo):
    in_maps = host_inputs(x, Wqkv, Wo, bo)
    r = run(in_maps)
    y = np.stack([r.results[b]["y"].reshape(C, H, W) for b in range(B)])
    return y.astype(np.float32)


if __name__ == "__main__":
    nc = build_nc()
    print("built ok:", len(nc.inst_map), "instructions")
